# revision 1
# baseline (speedup 1.0000x reference)
"""AA_GAT on 8 trn2 cores (self-contained).

Strategy: edges sharded by src-range (6272 nodes/core, 49 windows of 128
nodes). Host does layout only: sort/bucket edges per (core, window,
tgt-half), pad to 128-chunks, wrap arrays for the device. One per-core
node table H8 [NPAD, 640] bf16 (rotated by -core_base): cols 0:512 = 8
head embeddings, 512:528 = a_src/a_tgt attention dots.

Launch 1: node MLP (bn_stats LN, affine+relu fused into a post-transpose
scalar-engine activation) -> H8 in 4-tile batched writes; edge MLP ->
esc9 scores (same LN pattern); per window: merged dma_gather of tgt rows,
attention weights, one-hot S matmul segment-sum (num+den in PSUM),
elu -> xh, h_out = xh@out_W + out-layer alphas.  Host: concat h_out,
fold a_src[src]+e9 into P8.  Launch 2: out-layer edge pass (gather tgt
rows of HR bf16 only) + batched log_softmax at the end.
"""

import os
import numpy as np

import concourse.bass as bass
import concourse.mybir as mybir
import concourse.tile as tile
from concourse import bacc
from concourse.bass_utils import run_bass_kernel_spmd
from concourse.masks import make_identity

F32 = mybir.dt.float32
BF16 = mybir.dt.bfloat16
I16 = mybir.dt.int16
I32 = mybir.dt.int32
AF = mybir.ActivationFunctionType
OP = mybir.AluOpType
AX = mybir.AxisListType

N = 50000
E = 1_000_000
NODE_DIM = 16
EMB = 64
OUT = 64
HEADS = 8
EA_DIM = 8
SLOPE = 0.01
LN_EPS = 1e-5

NCORES = 8
NPC = 6272            # nodes per core (49*128)
NWIN = 49
NPAD = 50048          # 391*128
NT = NPAD // 128      # 391 node tiles
TSPLIT = 32768
HIB = NPAD - TSPLIT   # 17280 (hi gather base row)


def _wrap_slots(a):
    """[Emax,...] slot array -> [128, Emax/128, ...] with slot s at [s%128, s//128]."""
    if a.ndim == 1:
        return np.ascontiguousarray(a.reshape(-1, 128).T)
    return np.ascontiguousarray(a.reshape(-1, 128, a.shape[-1]).transpose(1, 0, 2))


def _wrap_idx(a):
    """int16 idx list [n] -> [128, n/16] (16-partition wrap replicated 8x)."""
    w = a.reshape(-1, 16).T  # [16, n/16]
    return np.ascontiguousarray(np.tile(w, (8, 1)))




GMAX = 1024  # dma_gather num_idxs HW limit (crashes ~2048)


def _gather(nc, out3, in_ap, idx_tile, total_n, elem, elem_step=None):
    """Split a gather into <=GMAX-idx calls. out3: [128, total_n//128, elem]."""
    for j0 in range(0, total_n, GMAX):
        n = min(GMAX, total_n - j0)
        kw = {}
        if elem_step is not None:
            kw["elem_step"] = elem_step
        nc.gpsimd.dma_gather(
            out_ap=out3[:, j0 // 128 : (j0 + n) // 128, :],
            in_ap=in_ap,
            idxs_ap=idx_tile[:, j0 // 16 : (j0 + n) // 16],
            num_idxs=n, num_idxs_reg=n, elem_size=elem, **kw)


def _prep(edge_index, edge_attr):
    src = np.asarray(edge_index[0]).astype(np.int64)
    tgt = np.asarray(edge_index[1]).astype(np.int64)
    ea = np.asarray(edge_attr).astype(np.float32)

    core_of = src // NPC
    win_of = (src % NPC) // 128

    # per-(core,window,half) edge lists
    buckets = {}
    nlo_max = 0
    nhi_max = 0
    for k in range(NCORES):
        mk = core_of == k
        idx_k = np.nonzero(mk)[0]
        rot = (tgt[idx_k] - k * NPC) % NPAD
        hi = rot >= TSPLIT
        w = win_of[idx_k]
        for ww in range(NWIN):
            mw = w == ww
            elo = idx_k[mw & ~hi]
            ehi = idx_k[mw & hi]
            buckets[(k, ww)] = (elo, ehi)
            nlo_max = max(nlo_max, len(elo))
            nhi_max = max(nhi_max, len(ehi))

    Klo = max(1, -(-nlo_max // 128))
    Khi = max(1, -(-nhi_max // 128))
    KT = Klo + Khi
    EMAXW = KT * 128
    Emax = NWIN * EMAXW

    per_core = []
    for k in range(NCORES):
        tgt16 = np.zeros(Emax, np.int16)
        srcw = np.zeros(Emax, np.int16)
        mask = np.zeros(Emax, np.float32)
        ea8 = np.zeros((Emax, EA_DIM), np.float32)
        srcg = np.zeros(Emax, np.int64)
        for ww in range(NWIN):
            elo, ehi = buckets[(k, ww)]
            base = ww * EMAXW
            for half, edges in ((0, elo), (1, ehi)):
                off = base + (0 if half == 0 else Klo * 128)
                n = len(edges)
                sl = slice(off, off + n)
                rot = (tgt[edges] - k * NPC) % NPAD
                tgt16[sl] = (rot if half == 0 else rot - HIB).astype(np.int16)
                srcw[sl] = (src[edges] - k * NPC - 128 * ww).astype(np.int16)
                mask[sl] = 1.0
                ea8[sl] = ea[edges]
                srcg[sl] = src[edges]
        # gather idx blocks per window
        tlo = np.zeros((NWIN, 128, Klo * 8), np.int16)
        thi = np.zeros((NWIN, 128, Khi * 8), np.int16)
        tsw = np.zeros((NWIN, 128, KT * 8), np.int16)
        for ww in range(NWIN):
            b = ww * EMAXW
            tlo[ww] = _wrap_idx(tgt16[b : b + Klo * 128])
            thi[ww] = _wrap_idx(tgt16[b + Klo * 128 : b + KT * 128])
            tsw[ww] = _wrap_idx(srcw[b : b + KT * 128])
        rotoffs = ((128 * np.arange(NT, dtype=np.int64) - k * NPC) % NPAD).astype(
            np.int32
        )
        # pad slots get srcw=-1 so the one-hot S has a zero column there
        # (replaces the explicit mask multiply)
        srcwf = np.where(mask > 0, srcw.astype(np.float32), -1.0)
        per_core.append(
            dict(
                tgtlo_idx=tlo,
                tgthi_idx=thi,
                srcw_idx=tsw,
                srcwin_f=_wrap_slots(srcwf),
                mask_w=_wrap_slots(mask),
                srcg=srcg,
                ea8T=np.ascontiguousarray(ea8.T),
                rotoffs=rotoffs.reshape(1, NT),
            )
        )
    return per_core, Klo, Khi


# ---------------------------------------------------------------- launch 1


NGRP = 98             # 97 groups of 4 node tiles + 1 group of 3 (391 total)
NPADX = NPAD + 512    # XTE extension so wrapped reads are contiguous


def _build_launch1(Klo, Khi):
    KT = Klo + Khi
    Emax = NWIN * KT * 128
    NCHUNK = NWIN * KT

    nc = bacc.Bacc("TRN2", target_bir_lowering=False, debug=False, num_devices=NCORES)
    din = lambda n, s, d=F32: nc.dram_tensor(n, s, d, kind="ExternalInput")
    XTE = din("XTE", [NODE_DIM + 1, NPADX], BF16)   # row 16 = ones; cols wrap
    WN = din("WN", [NODE_DIM + 1, EMB], BF16)       # row 16 = b_node
    LNPT = din("LNPT", [EMB, 2])                    # cols: g_node, beta_node
    LNET = din("LNET", [EMB, 2])                    # cols: g_edge, beta_edge
    WALL = din("WALL", [EMB, HEADS * OUT], BF16)
    WA = din("WA", [EMB, 16], BF16)                 # col i / 8+i = W_i@a_src/tgt_i
    AE9 = din("AE9", [EMB, 16], BF16)               # col j = a_edge unit j (9 used)
    WE = din("WE", [EA_DIM + 1, EMB], BF16)         # row 8 = b_edge
    OW = din("OW", [HEADS * OUT, OUT])
    OWA = din("OWA", [HEADS * OUT, 2])              # out_W @ [oa_src|oa_tgt]
    EA8T = din("EA8T", [EA_DIM + 1, Emax], BF16)    # row 8 = ones
    SRCWF = din("SRCWF", [128, Emax // 128], BF16)
    TLO = din("TLO", [NWIN, 128, Klo * 8], I16)
    THI = din("THI", [NWIN, 128, Khi * 8], I16)
    TSW = din("TSW", [NWIN, 128, KT * 8], I16)
    ROFF = din("ROFF", [1, NGRP], I32)

    H8 = nc.dram_tensor("H8", [NPAD, 640], BF16, kind="Internal")
    HOUTS = nc.dram_tensor("HOUTS", [NPC, 128], F32, kind="ExternalOutput")
    ESC9 = nc.dram_tensor("ESC9", [128, NCHUNK], F32, kind="ExternalOutput")

    with tile.TileContext(nc) as tc:
        with tc.tile_pool(name="const", bufs=1) as cpool:
            ident = cpool.tile([128, 128], F32)
            make_identity(nc, ident[:])
            iota = cpool.tile([128, 128], F32)
            nc.gpsimd.iota(iota[:], pattern=[[1, 128]], base=0, channel_multiplier=0,
                           allow_small_or_imprecise_dtypes=True)
            negone = cpool.tile([128, 1], F32)
            nc.gpsimd.memset(negone[:], -1.0)
            epst = cpool.tile([128, 1], F32)
            nc.gpsimd.memset(epst[:], LN_EPS)
            wn_sb = cpool.tile([NODE_DIM + 1, EMB], BF16)
            nc.sync.dma_start(wn_sb[:], WN[:])
            lnpt_sb = cpool.tile([EMB, 2], F32)
            nc.sync.dma_start(lnpt_sb[:], LNPT[:])
            lnet_sb = cpool.tile([EMB, 2], F32)
            nc.sync.dma_start(lnet_sb[:], LNET[:])
            wall_bf = cpool.tile([EMB, HEADS * OUT], BF16)
            nc.sync.dma_start(wall_bf[:], WALL[:])
            wa_bf = cpool.tile([EMB, 16], BF16)
            nc.sync.dma_start(wa_bf[:], WA[:])
            ae9_bf = cpool.tile([EMB, 16], BF16)
            nc.sync.dma_start(ae9_bf[:], AE9[:])
            we_sb = cpool.tile([EA_DIM + 1, EMB], BF16)
            nc.sync.dma_start(we_sb[:], WE[:])
            ow_sb = cpool.tile([128, 4, OUT], F32)
            nc.sync.dma_start(ow_sb[:], OW[:].rearrange("(a b) c -> b a c", b=128))
            owa_sb = cpool.tile([128, 4, 2], F32)
            nc.sync.dma_start(owa_sb[:], OWA[:].rearrange("(a b) c -> b a c", b=128))
            roff_sb = cpool.tile([1, NGRP], I32)
            nc.sync.dma_start(roff_sb[:], ROFF[:])

            # ---------------- stage A: x = MLP(X); H8, ATAB tables (rotated)
            # dest-major groups of 4 node tiles: table rows [512g, 512g+512)
            # written statically; source X cols read at dynamic (rotated)
            # offset from XTE (extended so reads never wrap).
            with (
                tc.tile_pool(name="sa", bufs=3) as sa,
                tc.tile_pool(name="sa_ps", bufs=2, space="PSUM") as sap,
                tc.tile_pool(name="sah_ps", bufs=2, space="PSUM") as sahp,
                tc.tile_pool(name="sat_ps", bufs=2, space="PSUM") as satp,
            ):
                for g in range(NGRP):
                    nt = 4 if g < NGRP - 1 else NT - 4 * (NGRP - 1)
                    rows = nt * 128
                    xt = sa.tile([NODE_DIM + 1, 512], BF16, tag="xt")
                    rv = nc.sync.alloc_register(f"roff{g}")
                    nc.sync.reg_load(rv, roff_sb[0:1, g : g + 1])
                    off = nc.sync.snap(rv, donate=True, min_val=0, max_val=NPAD - 1)
                    nc.sync.dma_start(xt[:, :rows], XTE[:, bass.ds(off, rows)])
                    y4 = sap.tile([128, 4, EMB], F32, tag="y4")
                    for t in range(nt):
                        nc.tensor.matmul(y4[:, t, :], lhsT=xt[:, 128 * t : 128 * (t + 1)],
                                         rhs=wn_sb[:], start=True, stop=True)
                    st6 = sa.tile([128, 4, 6], F32, tag="st6")
                    mv = sa.tile([128, 4, 2], F32, tag="mv")
                    for t in range(nt):
                        nc.vector.bn_stats(st6[:, t, :], y4[:, t, :])
                        nc.vector.bn_aggr(mv[:, t, :], st6[:, t, :])
                    iv = sa.tile([128, 4], F32, tag="iv")
                    nc.scalar.activation(iv[:, :nt], mv[:, :nt, 1], AF.Sqrt, bias=epst[:])
                    nc.vector.reciprocal(iv[:, :nt], iv[:, :nt])
                    hst = sa.tile([128, 4, 640], BF16, tag="hst")
                    for t in range(nt):
                        n_sb = sa.tile([128, EMB], F32, tag="nsb")
                        nc.vector.tensor_scalar(n_sb[:], y4[:, t, :], mv[:, t, 0:1],
                                                iv[:, t : t + 1], op0=OP.subtract,
                                                op1=OP.mult)
                        nT_ps = satp.tile([EMB, 128], F32, tag="nT")
                        nc.tensor.transpose(out=nT_ps[:], in_=n_sb[:], identity=ident[:])
                        xT_bf = sa.tile([EMB, 128], BF16, tag="xTbf")
                        nc.scalar.activation(xT_bf[:], nT_ps[:], AF.Relu,
                                             bias=lnpt_sb[:, 1:2], scale=lnpt_sb[:, 0:1])
                        h_ps = sahp.tile([128, HEADS * OUT], F32, tag="hps")
                        nc.tensor.matmul(h_ps[:], lhsT=xT_bf[:], rhs=wall_bf[:],
                                         start=True, stop=True)
                        nc.scalar.activation(hst[:, t, 0:512], h_ps[:], AF.Copy)
                        a_ps = satp.tile([128, 16], F32, tag="aps")
                        nc.tensor.matmul(a_ps[:], lhsT=xT_bf[:], rhs=wa_bf[:],
                                         start=True, stop=True)
                        nc.vector.tensor_copy(hst[:, t, 512:528], a_ps[:])
                    nc.sync.dma_start(
                        out=H8[512 * g : 512 * g + rows, :].rearrange(
                            "(t p) c -> p t c", p=128),
                        in_=hst[:, :nt, :])

            # ---------------- stage B: edge MLP -> esc9 (resident SBUF)
            esc9 = cpool.tile([128, NCHUNK, 9], F32)
            with (
                tc.tile_pool(name="sb", bufs=3) as sb,
                tc.tile_pool(name="sb_ps", bufs=2, space="PSUM") as sbp,
                tc.tile_pool(name="sbe_ps", bufs=2, space="PSUM") as sbep,
                tc.tile_pool(name="sbt_ps", bufs=3, space="PSUM") as sbtp,
            ):
                nmega = (NCHUNK + 7) // 8
                for m in range(nmega):
                    c0 = 8 * m
                    nch = min(8, NCHUNK - c0)
                    et = sb.tile([EA_DIM + 1, 8 * 128], BF16, tag="et")
                    nc.sync.dma_start(et[:, : nch * 128],
                                      EA8T[:, c0 * 128 : (c0 + nch) * 128])
                    y_ps = sbp.tile([128, 8, EMB], F32, tag="yps")
                    for c in range(nch):
                        nc.tensor.matmul(y_ps[:, c, :],
                                         lhsT=et[:, 128 * c : 128 * (c + 1)],
                                         rhs=we_sb[:], start=True, stop=True)
                    st6 = sb.tile([128, 8, 6], F32, tag="st6")
                    mv = sb.tile([128, 8, 2], F32, tag="mv")
                    for c in range(nch):
                        nc.vector.bn_stats(st6[:, c, :], y_ps[:, c, :])
                        nc.vector.bn_aggr(mv[:, c, :], st6[:, c, :])
                    iv = sb.tile([128, 8], F32, tag="iv")
                    nc.scalar.activation(iv[:, :nch], mv[:, :nch, 1], AF.Sqrt,
                                         bias=epst[:])
                    nc.vector.reciprocal(iv[:, :nch], iv[:, :nch])
                    e_ps = sbep.tile([128, 8, 16], F32, tag="eps")
                    for c in range(nch):
                        n_sb = sb.tile([128, EMB], F32, tag="nsb")
                        nc.vector.tensor_scalar(n_sb[:], y_ps[:, c, :], mv[:, c, 0:1],
                                                iv[:, c : c + 1], op0=OP.subtract,
                                                op1=OP.mult)
                        zT_ps = sbtp.tile([EMB, 128], F32, tag="zTps")
                        nc.tensor.transpose(out=zT_ps[:], in_=n_sb[:],
                                            identity=ident[:])
                        zT = sb.tile([EMB, 128], BF16, tag="zT")
                        nc.scalar.activation(zT[:], zT_ps[:], AF.Relu,
                                             bias=lnet_sb[:, 1:2],
                                             scale=lnet_sb[:, 0:1])
                        nc.tensor.matmul(e_ps[:, c, :], lhsT=zT[:], rhs=ae9_bf[:],
                                         start=True, stop=True)
                    nc.vector.tensor_copy(esc9[:, c0 : c0 + nch, :],
                                          e_ps[:, :nch, 0:9])
            # col 8 of esc9 -> DRAM for launch 2 (store as f32)
            with tc.tile_pool(name="e9", bufs=1) as e9p:
                e9 = e9p.tile([128, NCHUNK], F32)
                nc.vector.tensor_copy(e9[:], esc9[:, :, 8])
                nc.sync.dma_start(ESC9[:, :], e9[:])

            # ---------------- stage C: per-window edge pass (8 heads)
            srcwf_sb = cpool.tile([128, Emax // 128], BF16)
            nc.sync.dma_start(srcwf_sb[:], SRCWF[:])
            iota_w = cpool.tile([128, KT, 128], BF16)
            for c in range(KT):
                nc.vector.tensor_copy(iota_w[:, c, :], iota[:])
            with (
                tc.tile_pool(name="ec", bufs=2) as ec,
                tc.tile_pool(name="ecs", bufs=3) as ecs,
                tc.tile_pool(name="ec_ps", bufs=2, space="PSUM") as ecp,
                tc.tile_pool(name="ed_ps", bufs=1, space="PSUM") as edp,
                tc.tile_pool(name="et_ps", bufs=2, space="PSUM") as etp,
            ):
                for w in range(NWIN):
                    cw0 = w * KT
                    ilo = ec.tile([128, Klo * 8], I16, tag="ilo")
                    nc.sync.dma_start(ilo[:], TLO[w])
                    ihi = ec.tile([128, Khi * 8], I16, tag="ihi")
                    nc.sync.dma_start(ihi[:], THI[w])
                    isw = ec.tile([128, KT * 8], I16, tag="isw")
                    nc.sync.dma_start(isw[:], TSW[w])
                    g_src = ec.tile([128, KT, 128], BF16, tag="gsrc")
                    _gather(nc, g_src[:], H8[128 * w : 128 * (w + 1), 512:640], isw,
                            KT * 128, 128, elem_step=640)
                    G_lo = ec.tile([128, Klo, 640], BF16, tag="Glo")
                    _gather(nc, G_lo[:], H8[0:TSPLIT, :], ilo, Klo * 128, 640)
                    G_hi = ec.tile([128, Khi, 640], BF16, tag="Ghi")
                    _gather(nc, G_hi[:], H8[HIB:NPAD, :], ihi, Khi * 128, 640)
                    # scores
                    s8 = ecs.tile([128, KT, 8], F32, tag="s8")
                    nc.vector.tensor_tensor(out=s8[:, :Klo, :], in0=g_src[:, :Klo, 0:8],
                                            in1=G_lo[:, :, 520:528], op=OP.add)
                    nc.vector.tensor_tensor(out=s8[:, Klo:, :], in0=g_src[:, Klo:, 0:8],
                                            in1=G_hi[:, :, 520:528], op=OP.add)
                    nc.vector.tensor_tensor(out=s8[:], in0=s8[:],
                                            in1=esc9[:, cw0 : cw0 + KT, 0:8], op=OP.add)
                    lr = ecs.tile([128, KT, 8], F32, tag="lr")
                    nc.vector.tensor_scalar_mul(lr[:], s8[:], SLOPE)
                    nc.vector.tensor_tensor(out=s8[:], in0=s8[:], in1=lr[:], op=OP.max)
                    w8 = ecs.tile([128, KT, 8], BF16, tag="w8")
                    nc.scalar.activation(w8[:], s8[:], AF.Exp)
                    num_ps = ecp.tile([128, 512], F32, tag="num")
                    den_ps = edp.tile([128, 8], F32, tag="den")
                    S3 = ecs.tile([128, KT, 128], BF16, tag="S3")
                    nc.vector.tensor_tensor(
                        out=S3[:], in0=iota_w[:],
                        in1=srcwf_sb[:, cw0 : cw0 + KT].to_broadcast([128, KT, 128]),
                        op=OP.is_equal)
                    for c in range(KT):
                        G = G_lo[:, c, 0:512] if c < Klo else G_hi[:, c - Klo, 0:512]
                        V = ecs.tile([128, 512], BF16, tag="V")
                        nc.vector.tensor_tensor(
                            out=V[:].rearrange("p (i f) -> p i f", f=64),
                            in0=G.rearrange("p (i f) -> p i f", f=64),
                            in1=w8[:, c, :].to_broadcast([128, 8, 64]), op=OP.mult)
                        nc.tensor.matmul(num_ps[:], lhsT=S3[:, c, :], rhs=V[:],
                                         start=(c == 0), stop=(c == KT - 1))
                        nc.tensor.matmul(den_ps[:], lhsT=S3[:, c, :], rhs=w8[:, c, :],
                                         start=(c == 0), stop=(c == KT - 1))
                    # xh = elu(elu(num/den))
                    den = ecs.tile([128, 8], F32, tag="dens")
                    nc.vector.tensor_scalar(den[:], den_ps[:], 1e-16, None, op0=OP.add)
                    nc.vector.reciprocal(den[:], den[:])
                    xh = ecs.tile([128, 512], F32, tag="xh")
                    nc.vector.tensor_tensor(
                        out=xh[:].rearrange("p (i f) -> p i f", f=64),
                        in0=num_ps[:].rearrange("p (i f) -> p i f", f=64),
                        in1=den[:].to_broadcast([128, 8, 64]), op=OP.mult)
                    m0 = ecs.tile([128, 512], F32, tag="m0")
                    nc.vector.tensor_scalar_min(m0[:], xh[:], 0.0)
                    nc.scalar.activation(m0[:], m0[:], AF.Exp)
                    nc.scalar.activation(m0[:], m0[:], AF.Exp, bias=negone[:])
                    r0 = ecs.tile([128, 512], F32, tag="r0")
                    nc.scalar.activation(r0[:], xh[:], AF.Relu)
                    nc.vector.tensor_scalar(m0[:], m0[:], -1.0, None, op0=OP.add)
                    nc.vector.tensor_tensor(out=xh[:], in0=m0[:], in1=r0[:], op=OP.add)
                    # h_out slice + out-layer alphas
                    ho_ps = edp.tile([128, OUT], F32, tag="ho")
                    ao_ps = edp.tile([128, 2], F32, tag="ao")
                    for j in range(4):
                        xT_ps = etp.tile([128, 128], F32, tag="xTps2")
                        nc.tensor.transpose(out=xT_ps[:], in_=xh[:, 128 * j : 128 * (j + 1)],
                                            identity=ident[:])
                        xT = ecs.tile([128, 128], F32, tag="xT2")
                        nc.scalar.activation(xT[:], xT_ps[:], AF.Copy)
                        nc.tensor.matmul(ho_ps[:], lhsT=xT[:], rhs=ow_sb[:, j, :],
                                         start=(j == 0), stop=(j == 3))
                        nc.tensor.matmul(ao_ps[:], lhsT=xT[:], rhs=owa_sb[:, j, :],
                                         start=(j == 0), stop=(j == 3))
                    hrow = ecs.tile([128, 128], F32, tag="hrow")
                    nc.vector.memset(hrow[:, OUT + 2 :], 0.0)
                    nc.scalar.activation(hrow[:, 0:OUT], ho_ps[:], AF.Copy)
                    nc.scalar.activation(hrow[:, OUT : OUT + 2], ao_ps[:], AF.Copy)
                    nc.sync.dma_start(HOUTS[128 * w : 128 * (w + 1), :], hrow[:])
    nc.compile()
    return nc


# ---------------------------------------------------------------- launch 2


def _build_launch2(Klo, Khi):
    KT = Klo + Khi
    Emax = NWIN * KT * 128
    NCHUNK = NWIN * KT

    nc = bacc.Bacc("TRN2", target_bir_lowering=False, debug=False, num_devices=NCORES)
    din = lambda n, s, d=F32: nc.dram_tensor(n, s, d, kind="ExternalInput")
    HR = din("HR", [NPAD, 128], BF16)  # rotated [h_out(64) | a_src | a_tgt | pad]
    P8 = din("P8", [128, NCHUNK], BF16)  # e9 + a_src[src] per slot (host-folded)
    SRCWF = din("SRCWF", [128, Emax // 128])
    TLO = din("TLO", [NWIN, 128, Klo * 8], I16)
    THI = din("THI", [NWIN, 128, Khi * 8], I16)
    OUTT = nc.dram_tensor("OUTT", [NPC, OUT], F32, kind="ExternalOutput")

    with tile.TileContext(nc) as tc:
        with tc.tile_pool(name="const", bufs=1) as cpool:
            iota = cpool.tile([128, 128], F32)
            nc.gpsimd.iota(iota[:], pattern=[[1, 128]], base=0, channel_multiplier=0,
                           allow_small_or_imprecise_dtypes=True)
            p8_sb = cpool.tile([128, NCHUNK], BF16)
            nc.sync.dma_start(p8_sb[:], P8[:])
            srcwf_sb = cpool.tile([128, Emax // 128], F32)
            nc.sync.dma_start(srcwf_sb[:], SRCWF[:])
            hall = cpool.tile([128, NWIN, OUT], F32)
            with (
                tc.tile_pool(name="ec", bufs=2) as ec,
                tc.tile_pool(name="ecs", bufs=3) as ecs,
                tc.tile_pool(name="ec_ps", bufs=2, space="PSUM") as ecp,
                tc.tile_pool(name="ed_ps", bufs=2, space="PSUM") as edp,
            ):
                for w in range(NWIN):
                    cw0 = w * KT
                    ilo = ec.tile([128, Klo * 8], I16, tag="ilo")
                    nc.sync.dma_start(ilo[:], TLO[w])
                    ihi = ec.tile([128, Khi * 8], I16, tag="ihi")
                    nc.sync.dma_start(ihi[:], THI[w])
                    g_tlo = ec.tile([128, Klo, 128], BF16, tag="gtlo")
                    _gather(nc, g_tlo[:], HR[0:TSPLIT, :], ilo, Klo * 128, 128)
                    g_thi = ec.tile([128, Khi, 128], BF16, tag="gthi")
                    _gather(nc, g_thi[:], HR[HIB:NPAD, :], ihi, Khi * 128, 128)
                    s1 = ecs.tile([128, KT], F32, tag="s1")
                    nc.vector.tensor_tensor(out=s1[:, :Klo],
                                            in0=p8_sb[:, cw0 : cw0 + Klo],
                                            in1=g_tlo[:, :, 65], op=OP.add)
                    nc.vector.tensor_tensor(out=s1[:, Klo:],
                                            in0=p8_sb[:, cw0 + Klo : cw0 + KT],
                                            in1=g_thi[:, :, 65], op=OP.add)
                    lr = ecs.tile([128, KT], F32, tag="lr")
                    nc.vector.tensor_scalar_mul(lr[:], s1[:], SLOPE)
                    nc.vector.tensor_tensor(out=s1[:], in0=s1[:], in1=lr[:], op=OP.max)
                    w1 = ecs.tile([128, KT], BF16, tag="w1")
                    nc.scalar.activation(w1[:], s1[:], AF.Exp)
                    V = ecs.tile([128, KT, OUT], BF16, tag="V")
                    nc.vector.tensor_tensor(
                        out=V[:, :Klo, :], in0=g_tlo[:, :, 0:64],
                        in1=w1[:, :Klo].to_broadcast([128, Klo, 64]), op=OP.mult)
                    nc.vector.tensor_tensor(
                        out=V[:, Klo:, :], in0=g_thi[:, :, 0:64],
                        in1=w1[:, Klo:].to_broadcast([128, Khi, 64]), op=OP.mult)
                    num_ps = ecp.tile([128, OUT], F32, tag="num")
                    den_ps = edp.tile([128, 1], F32, tag="den")
                    for c in range(KT):
                        S = ecs.tile([128, 128], BF16, tag="S")
                        nc.vector.tensor_scalar(
                            S[:], iota[:], srcwf_sb[:, cw0 + c : cw0 + c + 1], None,
                            op0=OP.is_equal)
                        nc.tensor.matmul(num_ps[:], lhsT=S[:], rhs=V[:, c, :],
                                         start=(c == 0), stop=(c == KT - 1))
                        nc.tensor.matmul(den_ps[:], lhsT=S[:], rhs=w1[:, c : c + 1],
                                         start=(c == 0), stop=(c == KT - 1))
                    den = ecs.tile([128, 1], F32, tag="dens")
                    nc.vector.tensor_scalar(den[:], den_ps[:], 1e-16, None, op0=OP.add)
                    nc.vector.reciprocal(den[:], den[:])
                    h2 = ecs.tile([128, OUT], F32, tag="h2")
                    nc.vector.tensor_scalar(h2[:], num_ps[:], den[:], None, op0=OP.mult)
                    # elu -> hall[:, w, :]
                    m0 = ecs.tile([128, OUT], F32, tag="m0")
                    nc.vector.tensor_scalar_min(m0[:], h2[:], 0.0)
                    nc.scalar.activation(m0[:], m0[:], AF.Exp)
                    r0 = ecs.tile([128, OUT], F32, tag="r0")
                    nc.vector.tensor_scalar_max(r0[:], h2[:], 0.0)
                    nc.vector.scalar_tensor_tensor(out=hall[:, w, :], in0=m0[:],
                                                   scalar=-1.0, in1=r0[:],
                                                   op0=OP.add, op1=OP.add)
            # batched log_softmax over all windows (no max-sub: |h2| is small)
            with tc.tile_pool(name="fin", bufs=1) as fin:
                ex = fin.tile([128, NWIN, OUT], F32)
                nc.scalar.activation(ex[:], hall[:], AF.Exp)
                sm = fin.tile([128, NWIN], F32)
                nc.vector.tensor_reduce(sm[:], ex[:], axis=AX.X, op=OP.add)
                nc.scalar.activation(sm[:], sm[:], AF.Ln)
                res = fin.tile([128, NWIN, OUT], F32)
                nc.vector.tensor_tensor(out=res[:], in0=hall[:],
                                        in1=sm[:].to_broadcast([128, NWIN, OUT]),
                                        op=OP.subtract)
                nc.sync.dma_start(
                    OUTT[:].rearrange("(w p) f -> p w f", p=128), res[:])
    nc.compile()
    return nc


# ---------------------------------------------------------------- driver


def _make_inputs1(X, edge_attr, w_node, b_node, g_node, beta_node,
                  w_edge, b_edge, g_edge, beta_edge,
                  gat_W, gat_a, out_W, out_a, edge_index):
    import ml_dtypes
    bf = lambda a: np.ascontiguousarray(np.asarray(a, np.float32)).astype(
        ml_dtypes.bfloat16)
    X = np.asarray(X, np.float32)
    per_core, Klo, Khi = _prep(edge_index, edge_attr)

    # ---- shared (core-independent) inputs, host layout only
    Xp = np.zeros((NPAD, NODE_DIM + 1), np.float32)
    Xp[:N, :NODE_DIM] = X
    Xp[:, NODE_DIM] = 1.0
    XT = np.ascontiguousarray(Xp.T)
    XTE = bf(np.concatenate([XT, XT[:, :512]], 1))
    WN = bf(np.concatenate([np.asarray(w_node, np.float32),
                            np.asarray(b_node, np.float32)[None, :]], 0))
    WE = bf(np.concatenate([np.asarray(w_edge, np.float32),
                            np.asarray(b_edge, np.float32)[None, :]], 0))
    LNPT = np.stack([np.asarray(g_node, np.float32),
                     np.asarray(beta_node, np.float32)], 1)
    LNET = np.stack([np.asarray(g_edge, np.float32),
                     np.asarray(beta_edge, np.float32)], 1)
    gW = np.asarray(gat_W, np.float32)
    ga = np.asarray(gat_a, np.float32)
    oW = np.asarray(out_W, np.float32)
    oa = np.asarray(out_a, np.float32)
    WALL = bf(np.concatenate([gW[i] for i in range(HEADS)], 1))
    WA = np.zeros((EMB, 16), np.float32)
    for i in range(HEADS):
        WA[:, i] = gW[i] @ ga[i, :OUT]
        WA[:, 8 + i] = gW[i] @ ga[i, OUT : 2 * OUT]
    WA = bf(WA)
    AE9 = np.zeros((EMB, 16), np.float32)
    for i in range(HEADS):
        AE9[:, i] = ga[i, 2 * OUT :]
    AE9[:, 8] = oa[2 * OUT :]
    AE9 = bf(AE9)
    OWA = oW @ np.stack([oa[:OUT], oa[OUT : 2 * OUT]], 1)

    shared = dict(XTE=XTE, WN=WN, LNPT=LNPT, LNET=LNET, WALL=WALL,
                  WA=WA, AE9=AE9, WE=WE, OW=oW, OWA=OWA)

    in_maps = []
    for k in range(NCORES):
        pc = per_core[k]
        roff = ((512 * np.arange(NGRP, dtype=np.int64) + k * NPC) % NPAD).astype(
            np.int32).reshape(1, NGRP)
        in_maps.append({**{kk: np.ascontiguousarray(vv) for kk, vv in shared.items()},
                        "EA8T": bf(np.concatenate(
                            [pc["ea8T"], np.ones((1, pc["ea8T"].shape[1]), np.float32)], 0)),
                        "SRCWF": pc["srcwin_f"].astype(ml_dtypes.bfloat16),
                        "TLO": pc["tgtlo_idx"], "THI": pc["tgthi_idx"],
                        "TSW": pc["srcw_idx"], "ROFF": roff})
    return in_maps, per_core, Klo, Khi


def kernel(X, edge_attr, w_node, b_node, g_node, beta_node,
           w_edge, b_edge, g_edge, beta_edge,
           gat_W, gat_a, out_W, out_a,
           edge_index, matched_car_infra_nodes):
    in_maps, per_core, Klo, Khi = _make_inputs1(
        X, edge_attr, w_node, b_node, g_node, beta_node,
        w_edge, b_edge, g_edge, beta_edge,
        gat_W, gat_a, out_W, out_a, edge_index)
    import time as _time
    nc1 = _build_launch1(Klo, Khi)
    kernel.nc1 = nc1
    _t = _time.perf_counter()
    res1 = run_bass_kernel_spmd(nc1, in_maps, core_ids=list(range(NCORES)))
    kernel.wall1 = _time.perf_counter() - _t

    # host: assemble global HOUT and rotate per core
    import ml_dtypes
    HG = np.zeros((NPAD, 128), np.float32)
    for k in range(NCORES):
        lo = k * NPC
        hi = min((k + 1) * NPC, NPAD)
        HG[lo:hi] = res1.results[k]["HOUTS"][: hi - lo]

    nc2 = _build_launch2(Klo, Khi)
    in_maps2 = []
    for k in range(NCORES):
        pc = per_core[k]
        HR = np.ascontiguousarray(np.roll(HG, -k * NPC, axis=0)).astype(
            ml_dtypes.bfloat16)
        # fold a_src[src] into the per-slot edge score (host side)
        p8 = np.asarray(res1.results[k]["ESC9"], np.float32) + _wrap_slots(
            HG[pc["srcg"], 64].astype(np.float32))
        in_maps2.append({"HR": HR, "P8": p8.astype(ml_dtypes.bfloat16),
                         "SRCWF": pc["srcwin_f"],
                         "TLO": pc["tgtlo_idx"], "THI": pc["tgthi_idx"]})
    kernel.nc2 = nc2
    _t = _time.perf_counter()
    res2 = run_bass_kernel_spmd(nc2, in_maps2, core_ids=list(range(NCORES)))
    kernel.wall2 = _time.perf_counter() - _t

    out = np.zeros((N, OUT), np.float32)
    for k in range(NCORES):
        lo = k * NPC
        hi = min((k + 1) * NPC, N)
        out[lo:hi] = res2.results[k]["OUTT"][: hi - lo]
    return out



# revision 36
# speedup vs baseline: 2.0186x; 2.0186x over previous
"""AA_GAT on 8 trn2 cores (self-contained), v2.

Three launches; host does layout/gather only between launches.

L1: node MLP (nodes sharded 1/8 per core) + edge MLP (edges sharded by
    src-window). LN via Cholesky trick: y' = centered pre-LN output and
    u-columns come out of one matmul; var = sum(u^2)/64. beta=0 lets
    relu commute with the 1/sigma scale, so the only PSUM->SBUF bridge
    is a plain Relu; the iv scale is applied to the tiny outputs
    (adots 16 cols, esc 9 cols, x 64 cols once per node tile).
L2: layer-1 8-head edge pass per src window. Scores summed on PE from
    a host-transposed component table (esc8|asrc8|atgt8); exp on Act;
    per-edge value weighting V = w8 (x) xg via three engine paths
    (Act-replicate + DVE-stt / DVE tensor_tensor / Pool stt), one-hot
    segment-sum matmuls (host-prebuilt S3), elu(elu(.)), out-layer
    h_out = xh @ out_W + alpha dots.
L3: out-layer edge pass (same slot layout), then batched log_softmax.
"""

import numpy as np

import concourse.bass as bass
import concourse.mybir as mybir
import concourse.tile as tile
from concourse import bacc
from concourse.bass_utils import run_bass_kernel_spmd
from concourse.masks import make_identity

F32 = mybir.dt.float32
BF16 = mybir.dt.bfloat16
AF = mybir.ActivationFunctionType
OP = mybir.AluOpType
AX = mybir.AxisListType

N = 50000
E = 1_000_000
NODE_DIM = 16
EMB = 64
OUT = 64
HEADS = 8
EA_DIM = 8
SLOPE = 0.01
LN_EPS = 1e-5

NCORES = 8
NWIN = 49                 # windows (128 src nodes) per core
NPC = NWIN * 128          # 6272 nodes per core
NPN = NCORES * NPC        # 50176 padded node count
NWTOT = NCORES * NWIN     # 392 windows total
NDMA3 = 8                 # launch-3 one-hot chunks loaded via DMA

# L2 per-chunk V-path assignment (tuned): 'B' Act-replicate + DVE stt,
# 'A' DVE tensor_tensor broadcast, 'C' Pool stt broadcast.


def _vpaths(KT):
    # D = DVE tensor_tensor, P = Pool stt; alternate for 10:10 split
    order = "DPDPDPDPDPDPDPDPDPDP" * 4
    return [order[c % len(order)] for c in range(KT)]


# ------------------------------------------------------------------ host prep


def _prep(edge_index):
    """Degree-balanced node->window permutation and edge slot layout."""
    src = np.asarray(edge_index[0]).astype(np.int64)
    tgt = np.asarray(edge_index[1]).astype(np.int64)

    deg = np.bincount(src, minlength=N).astype(np.int64)
    # greedy: big-degree nodes first, into least-loaded window with space
    order = np.argsort(-deg, kind="stable")
    wload = np.zeros(NWTOT, np.int64)
    wcnt = np.zeros(NWTOT, np.int64)
    wnodes = [[] for _ in range(NWTOT)]
    import heapq

    heap = [(0, 0, w) for w in range(NWTOT)]
    heapq.heapify(heap)
    for n in order:
        while True:
            load, cnt, w = heapq.heappop(heap)
            if wcnt[w] < 128:
                break
        wnodes[w].append(n)
        wload[w] += deg[n]
        wcnt[w] += 1
        if wcnt[w] < 128:
            heapq.heappush(heap, (wload[w], wcnt[w], w))
    # order windows by load, snake-assign to cores for balance
    worder = np.argsort(-wload, kind="stable")
    core_wins = [[] for _ in range(NCORES)]
    fwd = True
    i = 0
    while i < NWTOT:
        rng = range(NCORES) if fwd else range(NCORES - 1, -1, -1)
        for k in rng:
            if i < NWTOT:
                core_wins[k].append(worder[i])
                i += 1
        fwd = not fwd
    # global permuted row id: core k, local window j, slot s
    pnode = np.full(N, -1, np.int64)
    origin = np.full(NPN, -1, np.int64)
    for k in range(NCORES):
        for j, w in enumerate(core_wins[k]):
            base = k * NPC + j * 128
            nodes = wnodes[w]
            for s, n in enumerate(nodes):
                pnode[n] = base + s
                origin[base + s] = n
    assert (pnode >= 0).all()

    psrc = pnode[src]
    ptgt = pnode[tgt]
    core_of = psrc // NPC
    win_of = (psrc % NPC) // 128
    srcw_of = psrc % 128

    KT = 0
    buckets = {}
    for k in range(NCORES):
        mk = core_of == k
        idx_k = np.nonzero(mk)[0]
        w = win_of[idx_k]
        for ww in range(NWIN):
            el = idx_k[w == ww]
            buckets[(k, ww)] = el
            KT = max(KT, (len(el) + 127) // 128)
    NS = KT * 128          # slots per window

    per_core = []
    for k in range(NCORES):
        eslot = np.full((NWIN, NS), -1, np.int64)     # edge id per slot
        for ww in range(NWIN):
            el = buckets[(k, ww)]
            eslot[ww, : len(el)] = el
        per_core.append(eslot)
    return per_core, pnode, origin, KT, srcw_of, ptgt


# ------------------------------------------------------------------ launch 1


def _build_launch1(NCHE, skip_node=False, max_blk=None, stop_at=99):
    """Node MLP (49 tiles) + edge MLP (NCHE chunks)."""
    nc = bacc.Bacc("TRN2", target_bir_lowering=False, debug=False,
                   num_devices=NCORES)
    din = lambda n, s, d=F32: nc.dram_tensor(n, s, d, kind="ExternalInput")
    XT17 = din("XT17", [NODE_DIM + 1, NPC], BF16)
    WNC = din("WNC", [NODE_DIM + 1, EMB + NODE_DIM + 1], BF16)
    WAB = din("WAB", [EMB, 16], BF16)
    AE9 = din("AE9", [128, 2, 16], BF16)     # [AE9;0] and [0;AE9] halves
    EAT9 = din("EAT9", [EA_DIM + 1, NCHE * 128], BF16)
    WEC = din("WEC", [EA_DIM + 1, EMB + EA_DIM + 1], BF16)

    XO = nc.dram_tensor("XO", [NPC, EMB], BF16, kind="ExternalOutput")
    AD = nc.dram_tensor("AD", [NPC, 16], F32, kind="ExternalOutput")
    ESC9 = nc.dram_tensor("ESC9", [128, NCHE, 9], F32, kind="ExternalOutput")

    KN = NODE_DIM + 1   # 17 u-cols (node)
    KE = EA_DIM + 1     # 9 u-cols (edge)

    with tile.TileContext(nc) as tc:
        with tc.tile_pool(name="const", bufs=1) as cpool:
            ident = cpool.tile([128, 128], BF16)
            make_identity(nc, ident[:])
            epst = cpool.tile([128, 1], F32)
            nc.gpsimd.memset(epst[:], LN_EPS)
            wnc_sb = cpool.tile([KN, EMB + KN], BF16)
            nc.sync.dma_start(wnc_sb[:], WNC[:])
            wab_sb = cpool.tile([EMB, 16], BF16)
            nc.sync.dma_start(wab_sb[:], WAB[:])
            ae9_sb = cpool.tile([128, 2, 16], BF16)
            nc.sync.dma_start(ae9_sb[:], AE9[:])
            wec_sb = cpool.tile([KE, EMB + KE], BF16)
            nc.sync.dma_start(wec_sb[:], WEC[:])

            # ------------- node MLP: 49 tiles, batch 4 for stats
            xout = cpool.tile([128, NWIN, EMB], BF16)
            adout = cpool.tile([128, NWIN, 16], F32)
            xt17 = cpool.tile([KN, NPC], BF16)
            nc.sync.dma_start(xt17[:], XT17[:])
            NG = 0 if skip_node else (NWIN + 3) // 4
            with (
                tc.tile_pool(name="na", bufs=3) as na,
                tc.tile_pool(name="na_ps", bufs=2, space="PSUM") as nap,
                tc.tile_pool(name="nt_ps", bufs=2, space="PSUM") as ntp,
                tc.tile_pool(name="nad_ps", bufs=2, space="PSUM") as nadp,
            ):
                for g in range(NG):
                    t0 = 4 * g
                    nt = min(4, NWIN - t0)
                    y4 = nap.tile([128, 4, EMB + KN], F32, tag="y4")
                    for t in range(nt):
                        nc.tensor.matmul(
                            y4[:, t, :],
                            lhsT=xt17[:, 128 * (t0 + t) : 128 * (t0 + t + 1)],
                            rhs=wnc_sb[:], start=True, stop=True)
                    u2 = na.tile([128, 4, KN], F32, tag="u2")
                    nc.scalar.activation(u2[:, :nt, :], y4[:, :nt, EMB:],
                                         AF.Square)
                    q = na.tile([128, 4], F32, tag="q")
                    nc.vector.tensor_reduce(q[:, :nt], u2[:, :nt, :],
                                            axis=AX.X, op=OP.add)
                    iv = na.tile([128, 4], F32, tag="iv")
                    nc.scalar.activation(iv[:, :nt], q[:, :nt], AF.Sqrt,
                                         bias=epst[:])
                    nc.vector.reciprocal(iv[:, :nt], iv[:, :nt])
                    for t in range(nt):
                        # x = max(iv*y'g, 0) directly into the table row
                        nc.vector.tensor_scalar(
                            xout[:, t0 + t, :], y4[:, t, :EMB],
                            iv[:, t : t + 1], 0.0, op0=OP.mult, op1=OP.max)
                        rT_ps = ntp.tile([EMB, 128], BF16, tag="rT")
                        nc.tensor.transpose(out=rT_ps[:],
                                            in_=xout[:, t0 + t, :],
                                            identity=ident[:])
                        rT = na.tile([EMB, 128], BF16, tag="rTs")
                        nc.scalar.activation(rT[:], rT_ps[:], AF.Copy)
                        a_ps = nadp.tile([128, 16], F32, tag="aps")
                        nc.tensor.matmul(a_ps[:], lhsT=rT[:], rhs=wab_sb[:],
                                         start=True, stop=True)
                        nc.vector.tensor_scalar(
                            adout[:, t0 + t, :], a_ps[:], 1.0, None,
                            op0=OP.mult)
            if skip_node:
                nc.gpsimd.memset(xout[:], 0.0)
                nc.gpsimd.memset(adout[:], 0.0)
            nc.sync.dma_start(
                XO[:].rearrange("(t p) c -> p t c", p=128), xout[:])
            nc.sync.dma_start(
                AD[:].rearrange("(t p) c -> p t c", p=128), adout[:])

            # ------------- edge MLP: blocks of 8 chunks (2 groups of 4)
            escb = cpool.tile([128, NCHE, 9], F32)
            NBLK = NCHE // 8 if max_blk is None else max_blk
            if max_blk is not None:
                nc.gpsimd.memset(escb[:], 0.0)
            with (
                tc.tile_pool(name="eld", bufs=2) as eld,
                tc.tile_pool(name="ea", bufs=4) as ea,
                tc.tile_pool(name="eb", bufs=3) as eb,
                tc.tile_pool(name="ea_ps", bufs=2, space="PSUM") as eap,
                tc.tile_pool(name="et_ps", bufs=2, space="PSUM") as etp,
                tc.tile_pool(name="ee_ps", bufs=2, space="PSUM") as eep,
            ):
                et = None
                for blk in range(NBLK):
                    c0 = 8 * blk
                    if blk % 2 == 0:
                        et = eld.tile([KE, 16 * 128], BF16, tag="et")
                        nb = min(16, NCHE - c0)
                        nc.sync.dma_start(
                            et[:, : nb * 128],
                            EAT9[:, c0 * 128 : (c0 + nb) * 128])
                    eo = (blk % 2) * 8 * 128
                    y4s = []
                    q8 = eb.tile([128, 8], F32, tag="q8")
                    for h in range(2):
                        y4 = eap.tile([128, 4, EMB + KE], F32,
                                      tag=f"y4{h}")
                        for c in range(4):
                            off = eo + 128 * (4 * h + c)
                            nc.tensor.matmul(
                                y4[:, c, :],
                                lhsT=et[:, off : off + 128],
                                rhs=wec_sb[:], start=True, stop=True)
                        if stop_at >= 2:
                            u2 = ea.tile([128, 4, KE], F32, tag="u2")
                            nc.scalar.activation(u2[:], y4[:, :, EMB:],
                                                 AF.Square)
                            nc.vector.tensor_reduce(
                                q8[:, 4 * h : 4 * h + 4],
                                u2[:], axis=AX.X, op=OP.add)
                        else:
                            nc.gpsimd.memset(q8[:, 4 * h : 4 * h + 4], 1.0)
                        y4s.append(y4)
                    iv8 = eb.tile([128, 8], F32, tag="iv8")
                    if stop_at >= 3:
                        nc.scalar.activation(iv8[:], q8[:], AF.Sqrt,
                                             bias=epst[:])
                        nc.vector.reciprocal(iv8[:], iv8[:])
                    else:
                        nc.gpsimd.memset(iv8[:], 1.0)
                    for h in range(2):
                        y4 = y4s[h]
                        n4 = ea.tile([128, 4, EMB], BF16, tag=f"n4{h}")
                        if stop_at < 4:
                            nc.gpsimd.memset(n4[:], 1.0)
                        for c in range(4 if stop_at >= 4 else 0):
                            ch = 4 * h + c
                            if (c0 + ch) % 8 < 5:
                                nc.vector.tensor_scalar(
                                    n4[:, c, :], y4[:, c, :EMB],
                                    iv8[:, ch : ch + 1], 0.0,
                                    op0=OP.mult, op1=OP.max)
                            else:
                                nc.scalar.activation(
                                    n4[:, c, :], y4[:, c, :EMB], AF.Relu,
                                    scale=iv8[:, ch : ch + 1])
                        zT_ps = etp.tile([128, 2, 128], BF16, tag="zT")
                        for j in range(2 if stop_at >= 5 else 0):
                            nc.tensor.transpose(
                                out=zT_ps[:, j, :],
                                in_=n4[:, 2 * j : 2 * j + 2, :].rearrange(
                                    "p a b -> p (a b)"),
                                identity=ident[:])
                        zT = ea.tile([128, 2, 128], BF16, tag=f"zTs{h}")
                        if stop_at < 6:
                            nc.gpsimd.memset(zT[:], 0.5)
                        elif h == 0:
                            nc.vector.tensor_scalar(zT[:], zT_ps[:], 1.0,
                                                    None, op0=OP.mult)
                        else:
                            nc.scalar.activation(zT[:], zT_ps[:], AF.Copy)
                        e_ps = eep.tile([128, 4, 16], F32, tag="eps")
                        for c in range(4 if stop_at >= 6 else 0):
                            nc.tensor.matmul(
                                e_ps[:, c, :],
                                lhsT=zT[:, c // 2, :],
                                rhs=ae9_sb[:, c % 2, :],
                                start=True, stop=True)
                        if stop_at >= 7:
                            nc.vector.tensor_scalar(
                                escb[:, c0 + 4 * h : c0 + 4 * h + 4, :],
                                e_ps[:, :, 0:9], 1.0, None, op0=OP.mult)
            nc.sync.dma_start(ESC9[:, :, :], escb[:])
    nc.compile()
    return nc


# ------------------------------------------------------------------ launch 2


def _build_launch2(KT):
    NS = KT * 128
    nc = bacc.Bacc("TRN2", target_bir_lowering=False, debug=False,
                   num_devices=NCORES)
    din = lambda n, s, d=F32: nc.dram_tensor(n, s, d, kind="ExternalInput")
    XG = din("XG", [NWIN, 128, KT, EMB], BF16)
    CMT = din("CMT", [NWIN, 24, NS], F32)
    S3H = din("S3H", [NWIN, 128, KT, 128], BF16)
    E24 = din("E24", [24, 8], F32)
    OWC = din("OWC", [128, 4, 66], BF16)    # [out_W | oa_src | oa_tgt] blocks
    WB4 = din("WB4", [128, 4, 128], BF16)   # block-diag gat_W head pairs
    HOUTS = nc.dram_tensor("HOUTS", [NPC, 66], F32, kind="ExternalOutput")

    vp = _vpaths(KT)

    with tile.TileContext(nc) as tc:
        with tc.tile_pool(name="const", bufs=1) as cpool:
            ident = cpool.tile([128, 128], BF16)
            make_identity(nc, ident[:])
            negone = cpool.tile([128, 1], F32)
            nc.gpsimd.memset(negone[:], -1.0)
            slp = cpool.tile([128, 1], F32)
            nc.gpsimd.memset(slp[:], SLOPE)
            e24_sb = cpool.tile([24, 8], F32)
            nc.sync.dma_start(e24_sb[:], E24[:])
            owc_sb = cpool.tile([128, 4, 66], BF16)
            nc.sync.dma_start(owc_sb[:], OWC[:])
            wb4_sb = cpool.tile([128, 4, 128], BF16)
            nc.sync.dma_start(wb4_sb[:], WB4[:])
            with (
                tc.tile_pool(name="w", bufs=2) as wp,
                tc.tile_pool(name="wv", bufs=3) as wv,
                tc.tile_pool(name="ws_ps", bufs=2, space="PSUM") as wsp,
                tc.tile_pool(name="wn_ps", bufs=2, space="PSUM") as wnp,
                tc.tile_pool(name="wt_ps", bufs=2, space="PSUM") as wtp,
            ):
                for w in range(NWIN):
                    xg = wp.tile([128, KT, EMB], BF16, tag="xg")
                    nc.sync.dma_start(xg[:], XG[w])
                    cmt = wp.tile([24, NS], F32, tag="cmt")
                    nc.sync.dma_start(cmt[:], CMT[w])
                    s3 = wp.tile([128, KT, 128], BF16, tag="s3")
                    nc.sync.dma_start(s3[:], S3H[w])
                    # scores: s8 = sum of components via PE
                    psu = wsp.tile([128, KT * 8 + 74], F32, tag="s8u")
                    s8_ps = psu[:, : KT * 8].rearrange(
                        "p (c i) -> p c i", i=8)
                    den_ps = psu[:, KT * 8 + 66 : KT * 8 + 74]
                    for c in range(KT):
                        nc.tensor.matmul(s8_ps[:, c, :],
                                         lhsT=cmt[:, 128 * c : 128 * (c + 1)],
                                         rhs=e24_sb[:], start=True, stop=True)
                    # w8 = exp(lrelu(s)) = max(exp(s), exp(0.01 s))
                    ex1 = wv.tile([128, KT, 8], BF16, tag="ex1")
                    nc.scalar.activation(ex1[:], s8_ps, AF.Exp)
                    ex2 = wv.tile([128, KT, 8], BF16, tag="ex2")
                    nc.scalar.activation(ex2[:], s8_ps, AF.Exp,
                                         scale=slp[:])
                    w8 = wv.tile([128, KT, 8], BF16, tag="w8")
                    nc.vector.tensor_tensor(out=w8[:], in0=ex1[:],
                                            in1=ex2[:], op=OP.max)
                    # V per chunk (DVE / Pool split) + one-hot matmuls
                    num_ps = wnp.tile([128, 512], F32, tag="num")
                    for c in range(KT):
                        V = wv.tile([128, HEADS, EMB], BF16, tag="V")
                        if vp[c] == "D":
                            nc.vector.tensor_tensor(
                                out=V[:],
                                in0=xg[:, c : c + 1, :].to_broadcast(
                                    [128, 8, EMB]),
                                in1=w8[:, c, :].to_broadcast([128, 8, EMB]),
                                op=OP.mult)
                        else:
                            nc.gpsimd.tensor_tensor(
                                out=V[:],
                                in0=xg[:, c : c + 1, :].to_broadcast(
                                    [128, 8, EMB]),
                                in1=w8[:, c, :].to_broadcast([128, 8, EMB]),
                                op=OP.mult)
                        nc.tensor.matmul(num_ps[:],
                                         lhsT=s3[:, c, :],
                                         rhs=V[:].rearrange(
                                             "p i f -> p (i f)"),
                                         start=(c == 0), stop=(c == KT - 1))
                        nc.tensor.matmul(den_ps, lhsT=s3[:, c, :],
                                         rhs=w8[:, c, :],
                                         start=(c == 0), stop=(c == KT - 1))
                    den = wv.tile([128, 8], F32, tag="dens")
                    nc.vector.tensor_scalar(den[:], den_ps, 1e-16, None,
                                            op0=OP.add)
                    nc.vector.reciprocal(den[:], den[:])
                    xh = wv.tile([128, 512], BF16, tag="xh")
                    nc.vector.tensor_tensor(
                        out=xh[:].rearrange("p (i f) -> p i f", f=EMB),
                        in0=num_ps[:].rearrange("p (i f) -> p i f", f=EMB),
                        in1=den[:].to_broadcast([128, 8, EMB]), op=OP.mult)
                    ho_ps = psu[:, KT * 8 : KT * 8 + 66]
                    # per-head W: transpose agg, W-matmul (stays f-major)
                    hh = wv.tile([128, 4, 128], BF16, tag="hh")
                    for j in range(4):
                        xT_ps = wtp.tile([128, 128], BF16, tag="xT")
                        nc.tensor.transpose(
                            out=xT_ps[:], in_=xh[:, 128 * j : 128 * (j + 1)],
                            identity=ident[:])
                        xT = wv.tile([128, 128], BF16, tag="xTs")
                        if j % 2 == 0:
                            nc.scalar.activation(xT[:], xT_ps[:], AF.Copy)
                        else:
                            nc.vector.tensor_scalar(xT[:], xT_ps[:], 1.0,
                                                    None, op0=OP.mult)
                        hT_ps = wtp.tile([128, 128], F32, tag="hT")
                        nc.tensor.matmul(hT_ps[:], lhsT=wb4_sb[:, j, :],
                                         rhs=xT[:], start=True, stop=True)
                        if j % 2 == 0:
                            nc.vector.tensor_scalar(hh[:, j, :], hT_ps[:],
                                                    1.0, None, op0=OP.mult)
                        else:
                            nc.scalar.activation(hh[:, j, :], hT_ps[:],
                                                 AF.Copy)
                    # elu(elu(.)) in f-major, batched over the 4 blocks
                    m0 = wv.tile([128, 512], BF16, tag="m0")
                    nc.vector.tensor_scalar_min(
                        m0[:], hh[:].rearrange("p a b -> p (a b)"), 0.0)
                    nc.scalar.activation(m0[:], m0[:], AF.Exp)
                    nc.scalar.activation(m0[:], m0[:], AF.Exp,
                                         bias=negone[:])
                    r0 = wv.tile([128, 512], BF16, tag="r0")
                    nc.vector.tensor_scalar(
                        r0[:], hh[:].rearrange("p a b -> p (a b)"), 0.0,
                        -1.0, op0=OP.max, op1=OP.add)
                    xh2 = wv.tile([128, 4, 128], BF16, tag="xh2")
                    nc.vector.tensor_tensor(
                        out=xh2[:].rearrange("p a b -> p (a b)"), in0=m0[:],
                        in1=r0[:], op=OP.add)
                    # out layer from f-major xh2 blocks
                    for j in range(4):
                        nc.tensor.matmul(ho_ps, lhsT=xh2[:, j, :],
                                         rhs=owc_sb[:, j, :],
                                         start=(j == 0), stop=(j == 3))
                    hrow = wv.tile([128, 66], F32, tag="hrow")
                    nc.scalar.activation(hrow[:], ho_ps, AF.Copy)
                    nc.sync.dma_start(HOUTS[128 * w : 128 * (w + 1), :],
                                      hrow[:])
    nc.compile()
    return nc


# ------------------------------------------------------------------ launch 3


def _build_launch3(KT):
    NS = KT * 128
    nc = bacc.Bacc("TRN2", target_bir_lowering=False, debug=False,
                   num_devices=NCORES)
    din = lambda n, s, d=F32: nc.dram_tensor(n, s, d, kind="ExternalInput")
    HG = din("HG", [NWIN, 128, KT, OUT], BF16)
    CM2 = din("CM2", [NWIN, 4, NS], F32)    # e9 | asrcO | atgtO | 0
    S3D = din("S3D", [NWIN, 128, NDMA3, 128], BF16)   # first NDMA3 chunks
    SRCWF = din("SRCWF", [128, NWIN, KT], F32)
    OUTT = nc.dram_tensor("OUTT", [NPC, OUT], F32, kind="ExternalOutput")

    with tile.TileContext(nc) as tc:
        with tc.tile_pool(name="const", bufs=1) as cpool:
            e4 = cpool.tile([4, 1], F32)
            nc.gpsimd.memset(e4[:], 1.0)
            slp = cpool.tile([128, 1], F32)
            nc.gpsimd.memset(slp[:], SLOPE)
            iota_bf = cpool.tile([128, 128], BF16)
            nc.gpsimd.iota(iota_bf[:], pattern=[[1, 128]], base=0,
                           channel_multiplier=0,
                           allow_small_or_imprecise_dtypes=True)
            srcwf = cpool.tile([128, NWIN, KT], F32)
            nc.sync.dma_start(srcwf[:], SRCWF[:])
            hall = cpool.tile([128, NWIN, OUT], F32)
            with (
                tc.tile_pool(name="w", bufs=2) as wp,
                tc.tile_pool(name="wv", bufs=3) as wv,
                tc.tile_pool(name="ws_ps", bufs=2, space="PSUM") as wsp,
                tc.tile_pool(name="wn_ps", bufs=2, space="PSUM") as wnp,
                tc.tile_pool(name="wd_ps", bufs=2, space="PSUM") as wdp,
            ):
                for w in range(NWIN):
                    hg = wp.tile([128, KT, OUT], BF16, tag="hg")
                    nc.sync.dma_start(hg[:], HG[w])
                    cm2 = wp.tile([4, NS], F32, tag="cm2")
                    nc.sync.dma_start(cm2[:], CM2[w])
                    s3 = wp.tile([128, KT, 128], BF16, tag="s3")
                    nc.sync.dma_start(s3[:, :NDMA3, :], S3D[w])
                    # build remaining one-hot chunks on DVE / Pool
                    for c in range(NDMA3, KT):
                        if c % 2 == 0:
                            nc.vector.tensor_scalar(
                                s3[:, c, :], iota_bf[:],
                                srcwf[:, w, c : c + 1], None,
                                op0=OP.is_equal)
                        else:
                            nc.gpsimd.tensor_scalar(
                                s3[:, c, :], iota_bf[:],
                                srcwf[:, w, c : c + 1], None,
                                op0=OP.is_equal)
                    s1_ps = wsp.tile([128, KT], F32, tag="s1")
                    for c in range(KT):
                        nc.tensor.matmul(s1_ps[:, c : c + 1],
                                         lhsT=cm2[:, 128 * c : 128 * (c + 1)],
                                         rhs=e4[:], start=True, stop=True)
                    # w1 = max(exp(s), exp(0.01 s))
                    ex1 = wv.tile([128, KT], BF16, tag="ex1")
                    nc.scalar.activation(ex1[:], s1_ps[:], AF.Exp)
                    ex2 = wv.tile([128, KT], BF16, tag="ex2")
                    nc.scalar.activation(ex2[:], s1_ps[:], AF.Exp,
                                         scale=slp[:])
                    w1 = wv.tile([128, KT], BF16, tag="w1")
                    nc.vector.tensor_tensor(out=w1[:], in0=ex1[:],
                                            in1=ex2[:], op=OP.max)
                    V1 = wv.tile([128, KT, OUT], BF16, tag="V1")
                    h3 = KT // 3
                    nc.vector.tensor_tensor(
                        out=V1[:, : 2 * h3, :], in0=hg[:, : 2 * h3, :],
                        in1=w1[:, : 2 * h3].to_broadcast(
                            [128, 2 * h3, OUT]), op=OP.mult)
                    nc.gpsimd.tensor_tensor(
                        out=V1[:, 2 * h3 :, :], in0=hg[:, 2 * h3 :, :],
                        in1=w1[:, 2 * h3 :].to_broadcast(
                            [128, KT - 2 * h3, OUT]),
                        op=OP.mult)
                    num_ps = wnp.tile([128, OUT], F32, tag="num")
                    den_ps = wdp.tile([128, 1], F32, tag="den")
                    for c in range(KT):
                        nc.tensor.matmul(num_ps[:], lhsT=s3[:, c, :],
                                         rhs=V1[:, c, :],
                                         start=(c == 0), stop=(c == KT - 1))
                        nc.tensor.matmul(den_ps[:], lhsT=s3[:, c, :],
                                         rhs=w1[:, c : c + 1],
                                         start=(c == 0), stop=(c == KT - 1))
                    den = wv.tile([128, 1], F32, tag="dens")
                    nc.vector.tensor_scalar(den[:], den_ps[:], 1e-16, None,
                                            op0=OP.add)
                    nc.vector.reciprocal(den[:], den[:])
                    h2 = wv.tile([128, OUT], F32, tag="h2")
                    nc.vector.tensor_scalar(h2[:], num_ps[:], den[:], None,
                                            op0=OP.mult)
                    m0 = wv.tile([128, OUT], F32, tag="m0")
                    nc.vector.tensor_scalar_min(m0[:], h2[:], 0.0)
                    nc.scalar.activation(m0[:], m0[:], AF.Exp)
                    r0 = wv.tile([128, OUT], F32, tag="r0")
                    nc.vector.tensor_scalar(r0[:], h2[:], 0.0, -1.0,
                                            op0=OP.max, op1=OP.add)
                    nc.vector.tensor_tensor(out=hall[:, w, :], in0=m0[:],
                                            in1=r0[:], op=OP.add)
            with tc.tile_pool(name="fin", bufs=1) as fin:
                ex = fin.tile([128, NWIN, OUT], F32)
                nc.scalar.activation(ex[:], hall[:], AF.Exp)
                sm = fin.tile([128, NWIN], F32)
                nc.vector.tensor_reduce(sm[:], ex[:], axis=AX.X, op=OP.add)
                nc.scalar.activation(sm[:], sm[:], AF.Ln)
                res = fin.tile([128, NWIN, OUT], F32)
                nc.vector.tensor_tensor(
                    out=res[:], in0=hall[:],
                    in1=sm[:].to_broadcast([128, NWIN, OUT]), op=OP.subtract)
                nc.sync.dma_start(
                    OUTT[:].rearrange("(w p) f -> p w f", p=128), res[:])
    nc.compile()
    return nc


# ------------------------------------------------------------------ driver


def kernel(X, edge_attr, w_node, b_node, g_node, beta_node,
           w_edge, b_edge, g_edge, beta_edge,
           gat_W, gat_a, out_W, out_a,
           edge_index, matched_car_infra_nodes):
    import ml_dtypes
    import time as _time

    bf = lambda a: np.ascontiguousarray(np.asarray(a, np.float32)).astype(
        ml_dtypes.bfloat16)
    f32 = lambda a: np.ascontiguousarray(np.asarray(a, np.float32))

    X = f32(X)
    ea = f32(edge_attr)
    w_node = f32(w_node); b_node = f32(b_node); g_node = f32(g_node)
    beta_node = f32(beta_node)
    w_edge = f32(w_edge); b_edge = f32(b_edge); g_edge = f32(g_edge)
    beta_edge = f32(beta_edge)
    gW = f32(gat_W); ga = f32(gat_a); oW = f32(out_W); oa = f32(out_a)
    assert np.abs(beta_node).max() < 1e-6 and np.abs(beta_edge).max() < 1e-6

    per_core, pnode, origin, KT, srcw_of, ptgt = _prep(edge_index)
    NS = KT * 128
    NCHE = NWIN * KT
    NCHE4 = ((NCHE + 15) // 16) * 16

    # ---- LN-folded weights (centered + Cholesky u-columns)
    def fold(Wb, bb, g, kdim):
        Wfull = np.concatenate([Wb, bb[None, :]], 0)          # [k, 64]
        m = Wfull.mean(axis=1)                                 # [k]
        Wc = Wfull - m[:, None]
        M = Wc @ Wc.T + 1e-10 * np.eye(kdim)
        B = np.linalg.cholesky(M) / np.sqrt(EMB)
        return np.concatenate([Wc * g[None, :], B], 1)         # [k, 64+k]

    WNC = bf(fold(w_node, b_node, g_node, NODE_DIM + 1))
    WEC = bf(fold(w_edge, b_edge, g_edge, EA_DIM + 1))
    WAB = np.zeros((EMB, 16), np.float32)
    for i in range(HEADS):
        WAB[:, i] = gW[i] @ ga[i, :OUT]
        WAB[:, 8 + i] = gW[i] @ ga[i, OUT : 2 * OUT]
    WAB = bf(WAB)
    AE9 = np.zeros((EMB, 16), np.float32)
    for i in range(HEADS):
        AE9[:, i] = ga[i, 2 * OUT :]
    AE9[:, 8] = oa[2 * OUT :]
    A2 = np.zeros((2, 128, 16), np.float32)
    A2[0, :EMB] = AE9
    A2[1, EMB:] = AE9
    AE9 = bf(A2.transpose(1, 0, 2))

    # ---- launch 1 inputs
    Xp = np.zeros((NPN, NODE_DIM + 1), np.float32)
    valid = origin >= 0
    Xp[valid, :NODE_DIM] = X[origin[valid]]
    Xp[:, NODE_DIM] = 1.0
    src = np.asarray(edge_index[0]).astype(np.int64)

    in_maps1 = []
    for k in range(NCORES):
        eslot = per_core[k]                                    # [NWIN, NS]
        eat = np.zeros((NCHE4 * 128, EA_DIM + 1), np.float32)
        es = eslot.reshape(-1)
        m = es >= 0
        eat[: NS * NWIN][m, :EA_DIM] = ea[es[m]]
        eat[: NS * NWIN][m, EA_DIM] = 1.0
        in_maps1.append(dict(
            XT17=bf(Xp[k * NPC : (k + 1) * NPC].T),
            WNC=WNC, WAB=WAB, AE9=AE9,
            EAT9=bf(eat.T), WEC=WEC))

    nc1 = _build_launch1(NCHE4)
    kernel.nc1 = nc1
    _t = _time.perf_counter()
    res1 = run_bass_kernel_spmd(nc1, in_maps1, core_ids=list(range(NCORES)))
    kernel.wall1 = _time.perf_counter() - _t

    # ---- host: assemble tables, gather per-slot inputs for launch 2
    XF = np.zeros((NPN, EMB), ml_dtypes.bfloat16)
    ADF = np.zeros((NPN, 16), np.float32)
    ESCF = []
    for k in range(NCORES):
        XF[k * NPC : (k + 1) * NPC] = res1.results[k]["XO"]
        ADF[k * NPC : (k + 1) * NPC] = res1.results[k]["AD"]
        # ESC9 [128, NCHE4, 9] -> slot-major [NWIN, NS, 9]
        e9 = np.asarray(res1.results[k]["ESC9"], np.float32)[:, :NCHE, :]
        e9 = e9.transpose(1, 0, 2).reshape(NWIN, NS, 9)
        ESCF.append(e9)

    # one-hot S3 per core (shared by launches 2 and 3)
    in_maps2 = []
    s3_cores = []
    for k in range(NCORES):
        eslot = per_core[k]
        es = eslot.reshape(NWIN, NS)
        m = es >= 0
        tgtrow = np.zeros((NWIN, NS), np.int64)
        tgtrow[m] = ptgt[es[m]]
        srcw = np.full((NWIN, NS), -1, np.int64)
        srcw[m] = srcw_of[es[m]]

        XGk = np.zeros((NWIN, NS, EMB), ml_dtypes.bfloat16)
        XGk[m] = XF[tgtrow[m]]
        CMTk = np.zeros((NWIN, 24, NS), np.float32)
        CMTk[:, 0:8, :] = ESCF[k][:, :, 0:8].transpose(0, 2, 1)
        srcrow_k = np.zeros((NWIN, NS), np.int64)
        # src row = core base + win*128 + srcw
        wid = np.arange(NWIN)[:, None]
        srcrow_k[m] = (k * NPC + (wid + np.zeros_like(srcw))[m] * 128
                       + srcw[m])
        asrc = np.zeros((NWIN, NS, 8), np.float32)
        asrc[m] = ADF[srcrow_k[m], 0:8]
        atgt = np.zeros((NWIN, NS, 8), np.float32)
        atgt[m] = ADF[tgtrow[m], 8:16]
        CMTk[:, 8:16, :] = asrc.transpose(0, 2, 1)
        CMTk[:, 16:24, :] = atgt.transpose(0, 2, 1)

        S3k = np.zeros((NWIN, NS, 128), ml_dtypes.bfloat16)
        ww, ss = np.nonzero(m)
        S3k[ww, ss, srcw[ww, ss]] = 1.0
        S3k = S3k.reshape(NWIN, KT, 128, 128).transpose(0, 2, 1, 3)
        s3_cores.append(np.ascontiguousarray(S3k))

        WB4 = np.zeros((128, 4, 128), np.float32)
        for j in range(4):
            for il in range(2):
                WB4[64 * il : 64 * il + 64, j,
                    64 * il : 64 * il + 64] = gW[2 * j + il]
        E24 = np.zeros((24, 8), np.float32)
        for i in range(8):
            E24[i, i] = 1.0
            E24[8 + i, i] = 1.0
            E24[16 + i, i] = 1.0
        OWC = np.zeros((512, 66), np.float32)
        OWC[:, 0:64] = oW
        OWC[:, 64] = oW @ oa[:OUT]
        OWC[:, 65] = oW @ oa[OUT : 2 * OUT]
        in_maps2.append(dict(
            XG=_slotmaj(XGk, KT, EMB),
            CMT=CMTk,
            S3H=s3_cores[k],
            E24=E24,
            OWC=bf(np.ascontiguousarray(
                OWC.reshape(4, 128, 66).transpose(1, 0, 2))),
            WB4=bf(WB4),
        ))

    nc2 = _build_launch2(KT)
    kernel.nc2 = nc2
    _t = _time.perf_counter()
    res2 = run_bass_kernel_spmd(nc2, in_maps2, core_ids=list(range(NCORES)))
    kernel.wall2 = _time.perf_counter() - _t

    # ---- host: assemble h_out table, gather for launch 3
    HF = np.zeros((NPN, 66), np.float32)
    for k in range(NCORES):
        HF[k * NPC : (k + 1) * NPC] = res2.results[k]["HOUTS"]
    HFb = HF[:, 0:64].astype(ml_dtypes.bfloat16)

    in_maps3 = []
    for k in range(NCORES):
        eslot = per_core[k]
        es = eslot.reshape(NWIN, NS)
        m = es >= 0
        tgtrow = np.zeros((NWIN, NS), np.int64)
        tgtrow[m] = ptgt[es[m]]
        srcw = np.full((NWIN, NS), -1, np.int64)
        srcw[m] = srcw_of[es[m]]
        wid = np.arange(NWIN)[:, None]
        srcrow_k = np.zeros((NWIN, NS), np.int64)
        srcrow_k[m] = (k * NPC + (wid + np.zeros_like(srcw))[m] * 128
                       + srcw[m])

        HGk = np.zeros((NWIN, NS, OUT), ml_dtypes.bfloat16)
        HGk[m] = HFb[tgtrow[m]]
        CM2k = np.zeros((NWIN, 4, NS), np.float32)
        CM2k[:, 0, :] = ESCF[k][:, :, 8]
        a_s = np.zeros((NWIN, NS), np.float32)
        a_s[m] = HF[srcrow_k[m], 64]
        a_t = np.zeros((NWIN, NS), np.float32)
        a_t[m] = HF[tgtrow[m], 65]
        CM2k[:, 1, :] = a_s
        CM2k[:, 2, :] = a_t
        srcwf_f = srcw.reshape(NWIN, KT, 128).transpose(2, 0, 1).astype(
            np.float32)
        in_maps3.append(dict(
            HG=_slotmaj(HGk, KT, OUT),
            CM2=CM2k,
            S3D=np.ascontiguousarray(s3_cores[k][:, :, :NDMA3, :]),
            SRCWF=np.ascontiguousarray(srcwf_f)))

    nc3 = _build_launch3(KT)
    kernel.nc3 = nc3
    _t = _time.perf_counter()
    res3 = run_bass_kernel_spmd(nc3, in_maps3, core_ids=list(range(NCORES)))
    kernel.wall3 = _time.perf_counter() - _t

    outp = np.zeros((NPN, OUT), np.float32)
    for k in range(NCORES):
        outp[k * NPC : (k + 1) * NPC] = res3.results[k]["OUTT"]
    out = np.zeros((N, OUT), np.float32)
    valid = origin >= 0
    out[origin[valid]] = outp[valid]
    return out


def _slotmaj(A, KT, F):
    """[NWIN, NS, F] with slot s=(c*128+p) -> [NWIN, 128, KT, F]."""
    NW = A.shape[0]
    return np.ascontiguousarray(
        A.reshape(NW, KT, 128, F).transpose(0, 2, 1, 3))


# revision 42
# speedup vs baseline: 2.2362x; 1.1078x over previous
"""AA_GAT on 8 trn2 cores (self-contained), v2.

Three launches; host does layout/gather only between launches.

L1: node MLP (nodes sharded 1/8 per core) + edge MLP (edges sharded by
    src-window). LN via Cholesky trick: y' = centered pre-LN output and
    u-columns come out of one matmul; var = sum(u^2)/64. beta=0 lets
    relu commute with the 1/sigma scale, so the only PSUM->SBUF bridge
    is a plain Relu; the iv scale is applied to the tiny outputs
    (adots 16 cols, esc 9 cols, x 64 cols once per node tile).
L2: layer-1 8-head edge pass per src window. Scores summed on PE from
    a host-transposed component table (esc8|asrc8|atgt8); exp on Act;
    per-edge value weighting V = w8 (x) xg via three engine paths
    (Act-replicate + DVE-stt / DVE tensor_tensor / Pool stt), one-hot
    segment-sum matmuls (host-prebuilt S3), elu(elu(.)), out-layer
    h_out = xh @ out_W + alpha dots.
L3: out-layer edge pass (same slot layout), then batched log_softmax.
"""

import numpy as np

import concourse.bass as bass
import concourse.mybir as mybir
import concourse.tile as tile
from concourse import bacc
from concourse.bass_utils import run_bass_kernel_spmd
from concourse.masks import make_identity

F32 = mybir.dt.float32
BF16 = mybir.dt.bfloat16
AF = mybir.ActivationFunctionType
OP = mybir.AluOpType
AX = mybir.AxisListType

N = 50000
E = 1_000_000
NODE_DIM = 16
EMB = 64
OUT = 64
HEADS = 8
EA_DIM = 8
SLOPE = 0.01
LN_EPS = 1e-5

NCORES = 8
NWIN = 49                 # windows (128 src nodes) per core
NPC = NWIN * 128          # 6272 nodes per core
NPN = NCORES * NPC        # 50176 padded node count
NWTOT = NCORES * NWIN     # 392 windows total
NDMA3 = 8                 # launch-3 one-hot chunks loaded via DMA

# L2 per-chunk V-path assignment (tuned): 'B' Act-replicate + DVE stt,
# 'A' DVE tensor_tensor broadcast, 'C' Pool stt broadcast.


def _vpaths(KT):
    # D = DVE tensor_tensor, P = Pool tensor_tensor; 12:8 split
    order = "DPDPDDPDPDDPDPDDPDPD" * 4
    return [order[c % len(order)] for c in range(KT)]


# ------------------------------------------------------------------ host prep


def _prep(edge_index):
    """Degree-balanced node->window permutation and edge slot layout."""
    src = np.asarray(edge_index[0]).astype(np.int64)
    tgt = np.asarray(edge_index[1]).astype(np.int64)

    deg = np.bincount(src, minlength=N).astype(np.int64)
    # greedy: big-degree nodes first, into least-loaded window with space
    order = np.argsort(-deg, kind="stable")
    wload = np.zeros(NWTOT, np.int64)
    wcnt = np.zeros(NWTOT, np.int64)
    wnodes = [[] for _ in range(NWTOT)]
    import heapq

    heap = [(0, 0, w) for w in range(NWTOT)]
    heapq.heapify(heap)
    for n in order:
        while True:
            load, cnt, w = heapq.heappop(heap)
            if wcnt[w] < 128:
                break
        wnodes[w].append(n)
        wload[w] += deg[n]
        wcnt[w] += 1
        if wcnt[w] < 128:
            heapq.heappush(heap, (wload[w], wcnt[w], w))
    # order windows by load, snake-assign to cores for balance
    worder = np.argsort(-wload, kind="stable")
    core_wins = [[] for _ in range(NCORES)]
    fwd = True
    i = 0
    while i < NWTOT:
        rng = range(NCORES) if fwd else range(NCORES - 1, -1, -1)
        for k in rng:
            if i < NWTOT:
                core_wins[k].append(worder[i])
                i += 1
        fwd = not fwd
    # global permuted row id: core k, local window j, slot s
    pnode = np.full(N, -1, np.int64)
    origin = np.full(NPN, -1, np.int64)
    for k in range(NCORES):
        for j, w in enumerate(core_wins[k]):
            base = k * NPC + j * 128
            nodes = wnodes[w]
            for s, n in enumerate(nodes):
                pnode[n] = base + s
                origin[base + s] = n
    assert (pnode >= 0).all()

    psrc = pnode[src]
    ptgt = pnode[tgt]
    core_of = psrc // NPC
    win_of = (psrc % NPC) // 128
    srcw_of = psrc % 128

    KT = 0
    buckets = {}
    for k in range(NCORES):
        mk = core_of == k
        idx_k = np.nonzero(mk)[0]
        w = win_of[idx_k]
        for ww in range(NWIN):
            el = idx_k[w == ww]
            buckets[(k, ww)] = el
            KT = max(KT, (len(el) + 127) // 128)
    NS = KT * 128          # slots per window

    per_core = []
    for k in range(NCORES):
        eslot = np.full((NWIN, NS), -1, np.int64)     # edge id per slot
        for ww in range(NWIN):
            el = buckets[(k, ww)]
            eslot[ww, : len(el)] = el
        per_core.append(eslot)
    return per_core, pnode, origin, KT, srcw_of, ptgt


# ------------------------------------------------------------------ launch 1


def _build_launch1(NCHE, skip_node=False, max_blk=None, stop_at=99):
    """Node MLP (49 tiles) + edge MLP (NCHE chunks)."""
    nc = bacc.Bacc("TRN2", target_bir_lowering=False, debug=False,
                   num_devices=NCORES)
    din = lambda n, s, d=F32: nc.dram_tensor(n, s, d, kind="ExternalInput")
    XT17 = din("XT17", [NODE_DIM + 1, NPC], BF16)
    WNC = din("WNC", [NODE_DIM + 1, EMB + NODE_DIM + 1], BF16)
    WAB = din("WAB", [EMB, 16], BF16)
    AE9 = din("AE9", [128, 2, 16], BF16)     # [AE9;0] and [0;AE9] halves
    EAT9 = din("EAT9", [EA_DIM + 1, NCHE * 128], BF16)
    WEC = din("WEC", [EA_DIM + 1, EMB + EA_DIM + 1], BF16)

    XO = nc.dram_tensor("XO", [NPC, EMB], BF16, kind="ExternalOutput")
    AD = nc.dram_tensor("AD", [NPC, 16], F32, kind="ExternalOutput")
    ESC9 = nc.dram_tensor("ESC9", [128, NCHE, 9], F32, kind="ExternalOutput")

    KN = NODE_DIM + 1   # 17 u-cols (node)
    KE = EA_DIM + 1     # 9 u-cols (edge)

    with tile.TileContext(nc) as tc:
        with tc.tile_pool(name="const", bufs=1) as cpool:
            ident = cpool.tile([128, 128], BF16)
            make_identity(nc, ident[:])
            epst = cpool.tile([128, 1], F32)
            nc.gpsimd.memset(epst[:], LN_EPS)
            wnc_sb = cpool.tile([KN, EMB + KN], BF16)
            nc.sync.dma_start(wnc_sb[:], WNC[:])
            wab_sb = cpool.tile([EMB, 16], BF16)
            nc.sync.dma_start(wab_sb[:], WAB[:])
            ae9_sb = cpool.tile([128, 2, 16], BF16)
            nc.sync.dma_start(ae9_sb[:], AE9[:])
            wec_sb = cpool.tile([KE, EMB + KE], BF16)
            nc.sync.dma_start(wec_sb[:], WEC[:])

            # ------------- node MLP: 49 tiles, batch 4 for stats
            xout = cpool.tile([128, NWIN, EMB], BF16)
            adout = cpool.tile([128, NWIN, 16], F32)
            xt17 = cpool.tile([KN, NPC], BF16)
            nc.sync.dma_start(xt17[:], XT17[:])
            NG = 0 if skip_node else (NWIN + 3) // 4
            with (
                tc.tile_pool(name="na", bufs=3) as na,
                tc.tile_pool(name="na_ps", bufs=2, space="PSUM") as nap,
                tc.tile_pool(name="nt_ps", bufs=2, space="PSUM") as ntp,
                tc.tile_pool(name="nad_ps", bufs=2, space="PSUM") as nadp,
            ):
                for g in range(NG):
                    t0 = 4 * g
                    nt = min(4, NWIN - t0)
                    y4 = nap.tile([128, 4, EMB + KN], F32, tag="y4")
                    for t in range(nt):
                        nc.tensor.matmul(
                            y4[:, t, :],
                            lhsT=xt17[:, 128 * (t0 + t) : 128 * (t0 + t + 1)],
                            rhs=wnc_sb[:], start=True, stop=True)
                    u2 = na.tile([128, 4, KN], F32, tag="u2")
                    nc.scalar.activation(u2[:, :nt, :], y4[:, :nt, EMB:],
                                         AF.Square)
                    q = na.tile([128, 4], F32, tag="q")
                    nc.vector.tensor_reduce(q[:, :nt], u2[:, :nt, :],
                                            axis=AX.X, op=OP.add)
                    iv = na.tile([128, 4], F32, tag="iv")
                    nc.scalar.activation(iv[:, :nt], q[:, :nt], AF.Sqrt,
                                         bias=epst[:])
                    nc.vector.reciprocal(iv[:, :nt], iv[:, :nt])
                    for t in range(nt):
                        # x = max(iv*y'g, 0) directly into the table row
                        nc.vector.tensor_scalar(
                            xout[:, t0 + t, :], y4[:, t, :EMB],
                            iv[:, t : t + 1], 0.0, op0=OP.mult, op1=OP.max)
                        rT_ps = ntp.tile([EMB, 128], BF16, tag="rT")
                        nc.tensor.transpose(out=rT_ps[:],
                                            in_=xout[:, t0 + t, :],
                                            identity=ident[:])
                        rT = na.tile([EMB, 128], BF16, tag="rTs")
                        nc.scalar.activation(rT[:], rT_ps[:], AF.Copy)
                        a_ps = nadp.tile([128, 16], F32, tag="aps")
                        nc.tensor.matmul(a_ps[:], lhsT=rT[:], rhs=wab_sb[:],
                                         start=True, stop=True)
                        nc.vector.tensor_scalar(
                            adout[:, t0 + t, :], a_ps[:], 1.0, None,
                            op0=OP.mult)
            if skip_node:
                nc.gpsimd.memset(xout[:], 0.0)
                nc.gpsimd.memset(adout[:], 0.0)
            nc.sync.dma_start(
                XO[:].rearrange("(t p) c -> p t c", p=128), xout[:])
            nc.sync.dma_start(
                AD[:].rearrange("(t p) c -> p t c", p=128), adout[:])

            # ------------- edge MLP: blocks of 8 chunks (2 groups of 4)
            escb = cpool.tile([128, NCHE, 9], F32)
            NBLK = NCHE // 8 if max_blk is None else max_blk
            if max_blk is not None:
                nc.gpsimd.memset(escb[:], 0.0)
            with (
                tc.tile_pool(name="eld", bufs=3) as eld,
                tc.tile_pool(name="ea", bufs=6) as ea,
                tc.tile_pool(name="eb", bufs=4) as eb,
                tc.tile_pool(name="ea_ps", bufs=2, space="PSUM") as eap,
                tc.tile_pool(name="et_ps", bufs=2, space="PSUM") as etp,
                tc.tile_pool(name="ee_ps", bufs=2, space="PSUM") as eep,
            ):
                et = None
                for blk in range(NBLK):
                    c0 = 8 * blk
                    if blk % 2 == 0:
                        et = eld.tile([KE, 16 * 128], BF16, tag="et")
                        nb = min(16, NCHE - c0)
                        nc.sync.dma_start(
                            et[:, : nb * 128],
                            EAT9[:, c0 * 128 : (c0 + nb) * 128])
                    eo = (blk % 2) * 8 * 128
                    y4s = []
                    q8 = eb.tile([128, 8], F32, tag="q8")
                    for h in range(2):
                        y4 = eap.tile([128, 4, EMB + KE], F32,
                                      tag=f"y4{h}")
                        for c in range(4):
                            off = eo + 128 * (4 * h + c)
                            nc.tensor.matmul(
                                y4[:, c, :],
                                lhsT=et[:, off : off + 128],
                                rhs=wec_sb[:], start=True, stop=True)
                        if stop_at >= 2:
                            u2 = ea.tile([128, 4, KE], F32, tag="u2")
                            nc.scalar.activation(u2[:], y4[:, :, EMB:],
                                                 AF.Square)
                            nc.vector.tensor_reduce(
                                q8[:, 4 * h : 4 * h + 4],
                                u2[:], axis=AX.X, op=OP.add)
                        else:
                            nc.gpsimd.memset(q8[:, 4 * h : 4 * h + 4], 1.0)
                        y4s.append(y4)
                    iv8 = eb.tile([128, 8], F32, tag="iv8")
                    if stop_at >= 3:
                        nc.scalar.activation(iv8[:], q8[:], AF.Sqrt,
                                             bias=epst[:])
                        nc.vector.reciprocal(iv8[:], iv8[:])
                    else:
                        nc.gpsimd.memset(iv8[:], 1.0)
                    for h in range(2):
                        y4 = y4s[h]
                        n4 = ea.tile([128, 4, EMB], BF16, tag=f"n4{h}")
                        if stop_at < 4:
                            nc.gpsimd.memset(n4[:], 1.0)
                        for c in range(4 if stop_at >= 4 else 0):
                            ch = 4 * h + c
                            if (c0 + ch) % 8 < 5:
                                nc.vector.tensor_scalar(
                                    n4[:, c, :], y4[:, c, :EMB],
                                    iv8[:, ch : ch + 1], 0.0,
                                    op0=OP.mult, op1=OP.max)
                            else:
                                nc.scalar.activation(
                                    n4[:, c, :], y4[:, c, :EMB], AF.Relu,
                                    scale=iv8[:, ch : ch + 1])
                        zT_ps = etp.tile([128, 2, 128], BF16, tag="zT")
                        for j in range(2 if stop_at >= 5 else 0):
                            nc.tensor.transpose(
                                out=zT_ps[:, j, :],
                                in_=n4[:, 2 * j : 2 * j + 2, :].rearrange(
                                    "p a b -> p (a b)"),
                                identity=ident[:])
                        zT = ea.tile([128, 2, 128], BF16, tag=f"zTs{h}")
                        if stop_at < 6:
                            nc.gpsimd.memset(zT[:], 0.5)
                        elif h == 0:
                            nc.vector.tensor_scalar(zT[:], zT_ps[:], 1.0,
                                                    None, op0=OP.mult)
                        else:
                            nc.scalar.activation(zT[:], zT_ps[:], AF.Copy)
                        e_ps = eep.tile([128, 4, 16], F32, tag="eps")
                        for c in range(4 if stop_at >= 6 else 0):
                            nc.tensor.matmul(
                                e_ps[:, c, :],
                                lhsT=zT[:, c // 2, :],
                                rhs=ae9_sb[:, c % 2, :],
                                start=True, stop=True)
                        if stop_at >= 7:
                            nc.vector.tensor_scalar(
                                escb[:, c0 + 4 * h : c0 + 4 * h + 4, :],
                                e_ps[:, :, 0:9], 1.0, None, op0=OP.mult)
            nc.sync.dma_start(ESC9[:, :, :], escb[:])
    nc.compile()
    return nc


# ------------------------------------------------------------------ launch 2


def _build_launch2(KT):
    NS = KT * 128
    nc = bacc.Bacc("TRN2", target_bir_lowering=False, debug=False,
                   num_devices=NCORES)
    din = lambda n, s, d=F32: nc.dram_tensor(n, s, d, kind="ExternalInput")
    XG = din("XG", [NWIN, 128, KT, EMB], BF16)
    CMT = din("CMT", [NWIN, 24, NS], F32)
    S3H = din("S3H", [NWIN, 128, KT, 128], BF16)
    E24 = din("E24", [24, 8], F32)
    OWC = din("OWC", [128, 4, 66], BF16)    # [out_W | oa_src | oa_tgt] blocks
    WB4 = din("WB4", [128, 4, 128], BF16)   # block-diag gat_W head pairs
    HOUTS = nc.dram_tensor("HOUTS", [NPC, 66], F32, kind="ExternalOutput")

    vp = _vpaths(KT)

    with tile.TileContext(nc) as tc:
        with tc.tile_pool(name="const", bufs=1) as cpool:
            ident = cpool.tile([128, 128], BF16)
            make_identity(nc, ident[:])
            negone = cpool.tile([128, 1], F32)
            nc.gpsimd.memset(negone[:], -1.0)
            slp = cpool.tile([128, 1], F32)
            nc.gpsimd.memset(slp[:], SLOPE)
            nslp = cpool.tile([128, 1], F32)
            nc.gpsimd.memset(nslp[:], -SLOPE)
            e24_sb = cpool.tile([24, 8], F32)
            nc.sync.dma_start(e24_sb[:], E24[:])
            owc_sb = cpool.tile([128, 4, 66], BF16)
            nc.sync.dma_start(owc_sb[:], OWC[:])
            wb4_sb = cpool.tile([128, 4, 128], BF16)
            nc.sync.dma_start(wb4_sb[:], WB4[:])
            with (
                tc.tile_pool(name="w", bufs=3) as wp,
                tc.tile_pool(name="wv", bufs=6) as wv,
                tc.tile_pool(name="ws_ps", bufs=2, space="PSUM") as wsp,
                tc.tile_pool(name="wn_ps", bufs=2, space="PSUM") as wnp,
                tc.tile_pool(name="wt_ps", bufs=2, space="PSUM") as wtp,
            ):
                for w in range(NWIN):
                    xg = wp.tile([128, KT, EMB], BF16, tag="xg")
                    nc.sync.dma_start(xg[:], XG[w])
                    cmt = wp.tile([24, NS], F32, tag="cmt")
                    nc.sync.dma_start(cmt[:], CMT[w])
                    s3 = wp.tile([128, KT, 128], BF16, tag="s3")
                    nc.sync.dma_start(s3[:], S3H[w])
                    # scores: s8 = sum of components via PE
                    psu = wsp.tile([128, KT * 8 + 74], F32, tag="s8u")
                    s8_ps = psu[:, : KT * 8].rearrange(
                        "p (c i) -> p c i", i=8)
                    den_ps = psu[:, KT * 8 + 66 : KT * 8 + 74]
                    for c in range(KT):
                        nc.tensor.matmul(s8_ps[:, c, :],
                                         lhsT=cmt[:, 128 * c : 128 * (c + 1)],
                                         rhs=e24_sb[:], start=True, stop=True)
                    # w8 = exp(lrelu(s)) = max(exp(s), exp(0.01 s))
                    ex1 = wv.tile([128, KT, 8], BF16, tag="ex1")
                    nc.scalar.activation(ex1[:], s8_ps, AF.Exp)
                    ex2 = wv.tile([128, KT, 8], BF16, tag="ex2")
                    nc.scalar.activation(ex2[:], s8_ps, AF.Exp,
                                         scale=slp[:])
                    w8 = wv.tile([128, KT, 8], BF16, tag="w8")
                    nc.vector.tensor_tensor(out=w8[:], in0=ex1[:],
                                            in1=ex2[:], op=OP.max)
                    # V per chunk (DVE / Pool split) + one-hot matmuls
                    num_ps = wnp.tile([128, 512], F32, tag="num")
                    for c in range(KT):
                        V = wv.tile([128, HEADS, EMB], BF16, tag="V")
                        if vp[c] == "D":
                            nc.vector.tensor_tensor(
                                out=V[:],
                                in0=xg[:, c : c + 1, :].to_broadcast(
                                    [128, 8, EMB]),
                                in1=w8[:, c, :].to_broadcast([128, 8, EMB]),
                                op=OP.mult)
                        else:
                            nc.gpsimd.tensor_tensor(
                                out=V[:],
                                in0=xg[:, c : c + 1, :].to_broadcast(
                                    [128, 8, EMB]),
                                in1=w8[:, c, :].to_broadcast([128, 8, EMB]),
                                op=OP.mult)
                        nc.tensor.matmul(num_ps[:],
                                         lhsT=s3[:, c, :],
                                         rhs=V[:].rearrange(
                                             "p i f -> p (i f)"),
                                         start=(c == 0), stop=(c == KT - 1))
                        nc.tensor.matmul(den_ps, lhsT=s3[:, c, :],
                                         rhs=w8[:, c, :],
                                         start=(c == 0), stop=(c == KT - 1))
                    den = wv.tile([128, 8], F32, tag="dens")
                    nc.vector.tensor_scalar(den[:], den_ps, 1e-16, None,
                                            op0=OP.add)
                    nc.vector.reciprocal(den[:], den[:])
                    xh = wv.tile([128, 512], BF16, tag="xh")
                    nc.vector.tensor_tensor(
                        out=xh[:].rearrange("p (i f) -> p i f", f=EMB),
                        in0=num_ps[:].rearrange("p (i f) -> p i f", f=EMB),
                        in1=den[:].to_broadcast([128, 8, EMB]), op=OP.mult)
                    ho_ps = psu[:, KT * 8 : KT * 8 + 66]
                    # per-head W: transpose agg, W-matmul (stays f-major)
                    hh = wv.tile([128, 4, 128], BF16, tag="hh")
                    for j in range(4):
                        xT_ps = wtp.tile([128, 128], BF16, tag="xT")
                        nc.tensor.transpose(
                            out=xT_ps[:], in_=xh[:, 128 * j : 128 * (j + 1)],
                            identity=ident[:])
                        xT = wv.tile([128, 128], BF16, tag="xTs")
                        if j % 2 == 0:
                            nc.scalar.activation(xT[:], xT_ps[:], AF.Copy)
                        else:
                            nc.vector.tensor_scalar(xT[:], xT_ps[:], 1.0,
                                                    None, op0=OP.mult)
                        hT_ps = wtp.tile([128, 128], F32, tag="hT")
                        nc.tensor.matmul(hT_ps[:], lhsT=wb4_sb[:, j, :],
                                         rhs=xT[:], start=True, stop=True)
                        if j % 2 == 0:
                            nc.vector.tensor_scalar(hh[:, j, :], hT_ps[:],
                                                    1.0, None, op0=OP.mult)
                        else:
                            nc.scalar.activation(hh[:, j, :], hT_ps[:],
                                                 AF.Copy)
                    # elu(elu(.)) in f-major, batched over the 4 blocks
                    m0 = wv.tile([128, 512], BF16, tag="m0")
                    nc.vector.tensor_scalar_min(
                        m0[:], hh[:].rearrange("p a b -> p (a b)"), 0.0)
                    nc.scalar.activation(m0[:], m0[:], AF.Exp)
                    nc.scalar.activation(m0[:], m0[:], AF.Exp,
                                         bias=negone[:])
                    r0 = wv.tile([128, 512], BF16, tag="r0")
                    nc.vector.tensor_scalar(
                        r0[:], hh[:].rearrange("p a b -> p (a b)"), 0.0,
                        -1.0, op0=OP.max, op1=OP.add)
                    xh2 = wv.tile([128, 4, 128], BF16, tag="xh2")
                    nc.vector.tensor_tensor(
                        out=xh2[:].rearrange("p a b -> p (a b)"), in0=m0[:],
                        in1=r0[:], op=OP.add)
                    # out layer from f-major xh2 blocks
                    for j in range(4):
                        nc.tensor.matmul(ho_ps, lhsT=xh2[:, j, :],
                                         rhs=owc_sb[:, j, :],
                                         start=(j == 0), stop=(j == 3))
                    hrow = wv.tile([128, 66], F32, tag="hrow")
                    nc.scalar.activation(hrow[:], ho_ps, AF.Copy)
                    nc.sync.dma_start(HOUTS[128 * w : 128 * (w + 1), :],
                                      hrow[:])
    nc.compile()
    return nc


# ------------------------------------------------------------------ launch 3


def _build_launch3(KT):
    NS = KT * 128
    nc = bacc.Bacc("TRN2", target_bir_lowering=False, debug=False,
                   num_devices=NCORES)
    din = lambda n, s, d=F32: nc.dram_tensor(n, s, d, kind="ExternalInput")
    HG = din("HG", [NWIN, 128, KT, OUT], BF16)
    CM2 = din("CM2", [NWIN, 4, NS], F32)    # e9 | asrcO | atgtO | 0
    S3D = din("S3D", [NWIN, 128, NDMA3, 128], BF16)   # first NDMA3 chunks
    SRCWF = din("SRCWF", [128, NWIN, KT], F32)
    OUTT = nc.dram_tensor("OUTT", [NPC, OUT], F32, kind="ExternalOutput")

    with tile.TileContext(nc) as tc:
        with tc.tile_pool(name="const", bufs=1) as cpool:
            e4 = cpool.tile([4, 1], F32)
            nc.gpsimd.memset(e4[:], 1.0)
            slp = cpool.tile([128, 1], F32)
            nc.gpsimd.memset(slp[:], SLOPE)
            iota_bf = cpool.tile([128, 128], BF16)
            nc.gpsimd.iota(iota_bf[:], pattern=[[1, 128]], base=0,
                           channel_multiplier=0,
                           allow_small_or_imprecise_dtypes=True)
            srcwf = cpool.tile([128, NWIN, KT], F32)
            nc.sync.dma_start(srcwf[:], SRCWF[:])
            hall = cpool.tile([128, NWIN, OUT], F32)
            with (
                tc.tile_pool(name="w", bufs=3) as wp,
                tc.tile_pool(name="wv", bufs=6) as wv,
                tc.tile_pool(name="ws_ps", bufs=2, space="PSUM") as wsp,
                tc.tile_pool(name="wn_ps", bufs=2, space="PSUM") as wnp,
                tc.tile_pool(name="wd_ps", bufs=2, space="PSUM") as wdp,
            ):
                for w in range(NWIN):
                    hg = wp.tile([128, KT, OUT], BF16, tag="hg")
                    nc.sync.dma_start(hg[:], HG[w])
                    cm2 = wp.tile([4, NS], F32, tag="cm2")
                    nc.sync.dma_start(cm2[:], CM2[w])
                    s3 = wp.tile([128, KT, 128], BF16, tag="s3")
                    nc.sync.dma_start(s3[:, :NDMA3, :], S3D[w])
                    # build remaining one-hot chunks on DVE / Pool
                    for c in range(NDMA3, KT):
                        if c % 2 == 0:
                            nc.vector.tensor_scalar(
                                s3[:, c, :], iota_bf[:],
                                srcwf[:, w, c : c + 1], None,
                                op0=OP.is_equal)
                        else:
                            nc.gpsimd.tensor_scalar(
                                s3[:, c, :], iota_bf[:],
                                srcwf[:, w, c : c + 1], None,
                                op0=OP.is_equal)
                    s1_ps = wsp.tile([128, KT], F32, tag="s1")
                    for c in range(KT):
                        nc.tensor.matmul(s1_ps[:, c : c + 1],
                                         lhsT=cm2[:, 128 * c : 128 * (c + 1)],
                                         rhs=e4[:], start=True, stop=True)
                    # w1 = max(exp(s), exp(0.01 s))
                    ex1 = wv.tile([128, KT], BF16, tag="ex1")
                    nc.scalar.activation(ex1[:], s1_ps[:], AF.Exp)
                    ex2 = wv.tile([128, KT], BF16, tag="ex2")
                    nc.scalar.activation(ex2[:], s1_ps[:], AF.Exp,
                                         scale=slp[:])
                    w1 = wv.tile([128, KT], BF16, tag="w1")
                    nc.vector.tensor_tensor(out=w1[:], in0=ex1[:],
                                            in1=ex2[:], op=OP.max)
                    V1 = wv.tile([128, KT, OUT], BF16, tag="V1")
                    h3 = KT // 3
                    nc.vector.tensor_tensor(
                        out=V1[:, : 2 * h3, :], in0=hg[:, : 2 * h3, :],
                        in1=w1[:, : 2 * h3].to_broadcast(
                            [128, 2 * h3, OUT]), op=OP.mult)
                    nc.gpsimd.tensor_tensor(
                        out=V1[:, 2 * h3 :, :], in0=hg[:, 2 * h3 :, :],
                        in1=w1[:, 2 * h3 :].to_broadcast(
                            [128, KT - 2 * h3, OUT]),
                        op=OP.mult)
                    num_ps = wnp.tile([128, OUT], F32, tag="num")
                    den_ps = wdp.tile([128, 1], F32, tag="den")
                    for c in range(KT):
                        nc.tensor.matmul(num_ps[:], lhsT=s3[:, c, :],
                                         rhs=V1[:, c, :],
                                         start=(c == 0), stop=(c == KT - 1))
                        nc.tensor.matmul(den_ps[:], lhsT=s3[:, c, :],
                                         rhs=w1[:, c : c + 1],
                                         start=(c == 0), stop=(c == KT - 1))
                    den = wv.tile([128, 1], F32, tag="dens")
                    nc.vector.tensor_scalar(den[:], den_ps[:], 1e-16, None,
                                            op0=OP.add)
                    nc.vector.reciprocal(den[:], den[:])
                    h2 = wv.tile([128, OUT], F32, tag="h2")
                    nc.vector.tensor_scalar(h2[:], num_ps[:], den[:], None,
                                            op0=OP.mult)
                    m0 = wv.tile([128, OUT], F32, tag="m0")
                    nc.vector.tensor_scalar_min(m0[:], h2[:], 0.0)
                    nc.scalar.activation(m0[:], m0[:], AF.Exp)
                    r0 = wv.tile([128, OUT], F32, tag="r0")
                    nc.vector.tensor_scalar(r0[:], h2[:], 0.0, -1.0,
                                            op0=OP.max, op1=OP.add)
                    nc.vector.tensor_tensor(out=hall[:, w, :], in0=m0[:],
                                            in1=r0[:], op=OP.add)
            with tc.tile_pool(name="fin", bufs=1) as fin:
                ex = fin.tile([128, NWIN, OUT], F32)
                nc.scalar.activation(ex[:], hall[:], AF.Exp)
                sm = fin.tile([128, NWIN], F32)
                nc.vector.tensor_reduce(sm[:], ex[:], axis=AX.X, op=OP.add)
                nc.scalar.activation(sm[:], sm[:], AF.Ln)
                res = fin.tile([128, NWIN, OUT], F32)
                nc.vector.tensor_tensor(
                    out=res[:], in0=hall[:],
                    in1=sm[:].to_broadcast([128, NWIN, OUT]), op=OP.subtract)
                nc.sync.dma_start(
                    OUTT[:].rearrange("(w p) f -> p w f", p=128), res[:])
    nc.compile()
    return nc


# ------------------------------------------------------------------ driver


def kernel(X, edge_attr, w_node, b_node, g_node, beta_node,
           w_edge, b_edge, g_edge, beta_edge,
           gat_W, gat_a, out_W, out_a,
           edge_index, matched_car_infra_nodes):
    import ml_dtypes
    import time as _time

    bf = lambda a: np.ascontiguousarray(np.asarray(a, np.float32)).astype(
        ml_dtypes.bfloat16)
    f32 = lambda a: np.ascontiguousarray(np.asarray(a, np.float32))

    X = f32(X)
    ea = f32(edge_attr)
    w_node = f32(w_node); b_node = f32(b_node); g_node = f32(g_node)
    beta_node = f32(beta_node)
    w_edge = f32(w_edge); b_edge = f32(b_edge); g_edge = f32(g_edge)
    beta_edge = f32(beta_edge)
    gW = f32(gat_W); ga = f32(gat_a); oW = f32(out_W); oa = f32(out_a)
    assert np.abs(beta_node).max() < 1e-6 and np.abs(beta_edge).max() < 1e-6

    per_core, pnode, origin, KT, srcw_of, ptgt = _prep(edge_index)
    NS = KT * 128
    NCHE = NWIN * KT
    NCHE4 = ((NCHE + 15) // 16) * 16

    # ---- LN-folded weights (centered + Cholesky u-columns)
    def fold(Wb, bb, g, kdim):
        Wfull = np.concatenate([Wb, bb[None, :]], 0)          # [k, 64]
        m = Wfull.mean(axis=1)                                 # [k]
        Wc = Wfull - m[:, None]
        M = Wc @ Wc.T + 1e-10 * np.eye(kdim)
        B = np.linalg.cholesky(M) / np.sqrt(EMB)
        return np.concatenate([Wc * g[None, :], B], 1)         # [k, 64+k]

    WNC = bf(fold(w_node, b_node, g_node, NODE_DIM + 1))
    WEC = bf(fold(w_edge, b_edge, g_edge, EA_DIM + 1))
    WAB = np.zeros((EMB, 16), np.float32)
    for i in range(HEADS):
        WAB[:, i] = gW[i] @ ga[i, :OUT]
        WAB[:, 8 + i] = gW[i] @ ga[i, OUT : 2 * OUT]
    WAB = bf(WAB)
    AE9 = np.zeros((EMB, 16), np.float32)
    for i in range(HEADS):
        AE9[:, i] = ga[i, 2 * OUT :]
    AE9[:, 8] = oa[2 * OUT :]
    A2 = np.zeros((2, 128, 16), np.float32)
    A2[0, :EMB] = AE9
    A2[1, EMB:] = AE9
    AE9 = bf(A2.transpose(1, 0, 2))

    # ---- launch 1 inputs
    Xp = np.zeros((NPN, NODE_DIM + 1), np.float32)
    valid = origin >= 0
    Xp[valid, :NODE_DIM] = X[origin[valid]]
    Xp[:, NODE_DIM] = 1.0
    src = np.asarray(edge_index[0]).astype(np.int64)

    in_maps1 = []
    for k in range(NCORES):
        eslot = per_core[k]                                    # [NWIN, NS]
        eat = np.zeros((NCHE4 * 128, EA_DIM + 1), np.float32)
        es = eslot.reshape(-1)
        m = es >= 0
        eat[: NS * NWIN][m, :EA_DIM] = ea[es[m]]
        eat[: NS * NWIN][m, EA_DIM] = 1.0
        in_maps1.append(dict(
            XT17=bf(Xp[k * NPC : (k + 1) * NPC].T),
            WNC=WNC, WAB=WAB, AE9=AE9,
            EAT9=bf(eat.T), WEC=WEC))

    nc1 = _build_launch1(NCHE4)
    kernel.nc1 = nc1
    _t = _time.perf_counter()
    res1 = run_bass_kernel_spmd(nc1, in_maps1, core_ids=list(range(NCORES)))
    kernel.wall1 = _time.perf_counter() - _t

    # ---- host: assemble tables, gather per-slot inputs for launch 2
    XF = np.zeros((NPN, EMB), ml_dtypes.bfloat16)
    ADF = np.zeros((NPN, 16), np.float32)
    ESCF = []
    for k in range(NCORES):
        XF[k * NPC : (k + 1) * NPC] = res1.results[k]["XO"]
        ADF[k * NPC : (k + 1) * NPC] = res1.results[k]["AD"]
        # ESC9 [128, NCHE4, 9] -> slot-major [NWIN, NS, 9]
        e9 = np.asarray(res1.results[k]["ESC9"], np.float32)[:, :NCHE, :]
        e9 = e9.transpose(1, 0, 2).reshape(NWIN, NS, 9)
        ESCF.append(e9)

    # one-hot S3 per core (shared by launches 2 and 3)
    in_maps2 = []
    s3_cores = []
    for k in range(NCORES):
        eslot = per_core[k]
        es = eslot.reshape(NWIN, NS)
        m = es >= 0
        tgtrow = np.zeros((NWIN, NS), np.int64)
        tgtrow[m] = ptgt[es[m]]
        srcw = np.full((NWIN, NS), -1, np.int64)
        srcw[m] = srcw_of[es[m]]

        XGk = np.zeros((NWIN, NS, EMB), ml_dtypes.bfloat16)
        XGk[m] = XF[tgtrow[m]]
        CMTk = np.zeros((NWIN, 24, NS), np.float32)
        CMTk[:, 0:8, :] = ESCF[k][:, :, 0:8].transpose(0, 2, 1)
        srcrow_k = np.zeros((NWIN, NS), np.int64)
        # src row = core base + win*128 + srcw
        wid = np.arange(NWIN)[:, None]
        srcrow_k[m] = (k * NPC + (wid + np.zeros_like(srcw))[m] * 128
                       + srcw[m])
        asrc = np.zeros((NWIN, NS, 8), np.float32)
        asrc[m] = ADF[srcrow_k[m], 0:8]
        atgt = np.zeros((NWIN, NS, 8), np.float32)
        atgt[m] = ADF[tgtrow[m], 8:16]
        CMTk[:, 8:16, :] = asrc.transpose(0, 2, 1)
        CMTk[:, 16:24, :] = atgt.transpose(0, 2, 1)

        S3k = np.zeros((NWIN, NS, 128), ml_dtypes.bfloat16)
        ww, ss = np.nonzero(m)
        S3k[ww, ss, srcw[ww, ss]] = 1.0
        S3k = S3k.reshape(NWIN, KT, 128, 128).transpose(0, 2, 1, 3)
        s3_cores.append(np.ascontiguousarray(S3k))

        WB4 = np.zeros((128, 4, 128), np.float32)
        for j in range(4):
            for il in range(2):
                WB4[64 * il : 64 * il + 64, j,
                    64 * il : 64 * il + 64] = gW[2 * j + il]
        E24 = np.zeros((24, 8), np.float32)
        for i in range(8):
            E24[i, i] = 1.0
            E24[8 + i, i] = 1.0
            E24[16 + i, i] = 1.0
        OWC = np.zeros((512, 66), np.float32)
        OWC[:, 0:64] = oW
        OWC[:, 64] = oW @ oa[:OUT]
        OWC[:, 65] = oW @ oa[OUT : 2 * OUT]
        in_maps2.append(dict(
            XG=_slotmaj(XGk, KT, EMB),
            CMT=CMTk,
            S3H=s3_cores[k],
            E24=E24,
            OWC=bf(np.ascontiguousarray(
                OWC.reshape(4, 128, 66).transpose(1, 0, 2))),
            WB4=bf(WB4),
        ))

    nc2 = _build_launch2(KT)
    kernel.nc2 = nc2
    _t = _time.perf_counter()
    res2 = run_bass_kernel_spmd(nc2, in_maps2, core_ids=list(range(NCORES)))
    kernel.wall2 = _time.perf_counter() - _t

    # ---- host: assemble h_out table, gather for launch 3
    HF = np.zeros((NPN, 66), np.float32)
    for k in range(NCORES):
        HF[k * NPC : (k + 1) * NPC] = res2.results[k]["HOUTS"]
    HFb = HF[:, 0:64].astype(ml_dtypes.bfloat16)

    in_maps3 = []
    for k in range(NCORES):
        eslot = per_core[k]
        es = eslot.reshape(NWIN, NS)
        m = es >= 0
        tgtrow = np.zeros((NWIN, NS), np.int64)
        tgtrow[m] = ptgt[es[m]]
        srcw = np.full((NWIN, NS), -1, np.int64)
        srcw[m] = srcw_of[es[m]]
        wid = np.arange(NWIN)[:, None]
        srcrow_k = np.zeros((NWIN, NS), np.int64)
        srcrow_k[m] = (k * NPC + (wid + np.zeros_like(srcw))[m] * 128
                       + srcw[m])

        HGk = np.zeros((NWIN, NS, OUT), ml_dtypes.bfloat16)
        HGk[m] = HFb[tgtrow[m]]
        CM2k = np.zeros((NWIN, 4, NS), np.float32)
        CM2k[:, 0, :] = ESCF[k][:, :, 8]
        a_s = np.zeros((NWIN, NS), np.float32)
        a_s[m] = HF[srcrow_k[m], 64]
        a_t = np.zeros((NWIN, NS), np.float32)
        a_t[m] = HF[tgtrow[m], 65]
        CM2k[:, 1, :] = a_s
        CM2k[:, 2, :] = a_t
        srcwf_f = srcw.reshape(NWIN, KT, 128).transpose(2, 0, 1).astype(
            np.float32)
        in_maps3.append(dict(
            HG=_slotmaj(HGk, KT, OUT),
            CM2=CM2k,
            S3D=np.ascontiguousarray(s3_cores[k][:, :, :NDMA3, :]),
            SRCWF=np.ascontiguousarray(srcwf_f)))

    nc3 = _build_launch3(KT)
    kernel.nc3 = nc3
    _t = _time.perf_counter()
    res3 = run_bass_kernel_spmd(nc3, in_maps3, core_ids=list(range(NCORES)))
    kernel.wall3 = _time.perf_counter() - _t

    outp = np.zeros((NPN, OUT), np.float32)
    for k in range(NCORES):
        outp[k * NPC : (k + 1) * NPC] = res3.results[k]["OUTT"]
    out = np.zeros((N, OUT), np.float32)
    valid = origin >= 0
    out[origin[valid]] = outp[valid]
    return out


def _slotmaj(A, KT, F):
    """[NWIN, NS, F] with slot s=(c*128+p) -> [NWIN, 128, KT, F]."""
    NW = A.shape[0]
    return np.ascontiguousarray(
        A.reshape(NW, KT, 128, F).transpose(0, 2, 1, 3))


# revision 44
# speedup vs baseline: 2.2478x; 1.0052x over previous
"""AA_GAT on 8 trn2 cores (self-contained), v2.

Three launches; host does layout/gather only between launches.

L1: node MLP (nodes sharded 1/8 per core) + edge MLP (edges sharded by
    src-window). LN via Cholesky trick: y' = centered pre-LN output and
    u-columns come out of one matmul; var = sum(u^2)/64. beta=0 lets
    relu commute with the 1/sigma scale, so the only PSUM->SBUF bridge
    is a plain Relu; the iv scale is applied to the tiny outputs
    (adots 16 cols, esc 9 cols, x 64 cols once per node tile).
L2: layer-1 8-head edge pass per src window. Scores summed on PE from
    a host-transposed component table (esc8|asrc8|atgt8); exp on Act;
    per-edge value weighting V = w8 (x) xg via three engine paths
    (Act-replicate + DVE-stt / DVE tensor_tensor / Pool stt), one-hot
    segment-sum matmuls (host-prebuilt S3), elu(elu(.)), out-layer
    h_out = xh @ out_W + alpha dots.
L3: out-layer edge pass (same slot layout), then batched log_softmax.
"""

import numpy as np

import concourse.bass as bass
import concourse.mybir as mybir
import concourse.tile as tile
from concourse import bacc
from concourse.bass_utils import run_bass_kernel_spmd
from concourse.masks import make_identity

F32 = mybir.dt.float32
BF16 = mybir.dt.bfloat16
AF = mybir.ActivationFunctionType
OP = mybir.AluOpType
AX = mybir.AxisListType

N = 50000
E = 1_000_000
NODE_DIM = 16
EMB = 64
OUT = 64
HEADS = 8
EA_DIM = 8
SLOPE = 0.01
LN_EPS = 1e-5

NCORES = 8
NWIN = 49                 # windows (128 src nodes) per core
NPC = NWIN * 128          # 6272 nodes per core
NPN = NCORES * NPC        # 50176 padded node count
NWTOT = NCORES * NWIN     # 392 windows total
NDMA3 = 8                 # launch-3 one-hot chunks loaded via DMA

# L2 per-chunk V-path assignment (tuned): 'B' Act-replicate + DVE stt,
# 'A' DVE tensor_tensor broadcast, 'C' Pool stt broadcast.


def _vpaths(KT):
    # D = DVE tensor_tensor, P = Pool tensor_tensor; 12:8 split
    order = "DPDPDDPDPDDPDPDDPDPD" * 4
    return [order[c % len(order)] for c in range(KT)]


# ------------------------------------------------------------------ host prep


def _prep(edge_index):
    """Degree-balanced node->window permutation and edge slot layout."""
    src = np.asarray(edge_index[0]).astype(np.int64)
    tgt = np.asarray(edge_index[1]).astype(np.int64)

    deg = np.bincount(src, minlength=N).astype(np.int64)
    # greedy: big-degree nodes first, into least-loaded window with space
    order = np.argsort(-deg, kind="stable")
    wload = np.zeros(NWTOT, np.int64)
    wcnt = np.zeros(NWTOT, np.int64)
    wnodes = [[] for _ in range(NWTOT)]
    import heapq

    heap = [(0, 0, w) for w in range(NWTOT)]
    heapq.heapify(heap)
    for n in order:
        while True:
            load, cnt, w = heapq.heappop(heap)
            if wcnt[w] < 128:
                break
        wnodes[w].append(n)
        wload[w] += deg[n]
        wcnt[w] += 1
        if wcnt[w] < 128:
            heapq.heappush(heap, (wload[w], wcnt[w], w))
    # order windows by load, snake-assign to cores for balance
    worder = np.argsort(-wload, kind="stable")
    core_wins = [[] for _ in range(NCORES)]
    fwd = True
    i = 0
    while i < NWTOT:
        rng = range(NCORES) if fwd else range(NCORES - 1, -1, -1)
        for k in rng:
            if i < NWTOT:
                core_wins[k].append(worder[i])
                i += 1
        fwd = not fwd
    # global permuted row id: core k, local window j, slot s
    pnode = np.full(N, -1, np.int64)
    origin = np.full(NPN, -1, np.int64)
    for k in range(NCORES):
        for j, w in enumerate(core_wins[k]):
            base = k * NPC + j * 128
            nodes = wnodes[w]
            for s, n in enumerate(nodes):
                pnode[n] = base + s
                origin[base + s] = n
    assert (pnode >= 0).all()

    psrc = pnode[src]
    ptgt = pnode[tgt]
    core_of = psrc // NPC
    win_of = (psrc % NPC) // 128
    srcw_of = psrc % 128

    KT = 0
    buckets = {}
    for k in range(NCORES):
        mk = core_of == k
        idx_k = np.nonzero(mk)[0]
        w = win_of[idx_k]
        for ww in range(NWIN):
            el = idx_k[w == ww]
            buckets[(k, ww)] = el
            KT = max(KT, (len(el) + 127) // 128)
    NS = KT * 128          # slots per window

    per_core = []
    for k in range(NCORES):
        eslot = np.full((NWIN, NS), -1, np.int64)     # edge id per slot
        for ww in range(NWIN):
            el = buckets[(k, ww)]
            eslot[ww, : len(el)] = el
        per_core.append(eslot)
    return per_core, pnode, origin, KT, srcw_of, ptgt


# ------------------------------------------------------------------ launch 1


def _build_launch1(NCHE, skip_node=False, max_blk=None, stop_at=99):
    """Node MLP (49 tiles) + edge MLP (NCHE chunks)."""
    nc = bacc.Bacc("TRN2", target_bir_lowering=False, debug=False,
                   num_devices=NCORES)
    din = lambda n, s, d=F32: nc.dram_tensor(n, s, d, kind="ExternalInput")
    XT17 = din("XT17", [NODE_DIM + 1, NPC], BF16)
    WNC = din("WNC", [NODE_DIM + 1, EMB + NODE_DIM + 1], BF16)
    WAB = din("WAB", [EMB, 16], BF16)
    AE9 = din("AE9", [128, 2, 16], BF16)     # [AE9;0] and [0;AE9] halves
    EAT9 = din("EAT9", [EA_DIM + 1, NCHE * 128], BF16)
    WEC = din("WEC", [EA_DIM + 1, EMB + EA_DIM + 1], BF16)

    XO = nc.dram_tensor("XO", [NPC, EMB], BF16, kind="ExternalOutput")
    AD = nc.dram_tensor("AD", [NPC, 16], F32, kind="ExternalOutput")
    ESC9 = nc.dram_tensor("ESC9", [128, NCHE, 9], F32, kind="ExternalOutput")

    KN = NODE_DIM + 1   # 17 u-cols (node)
    KE = EA_DIM + 1     # 9 u-cols (edge)

    with tile.TileContext(nc) as tc:
        with tc.tile_pool(name="const", bufs=1) as cpool:
            ident = cpool.tile([128, 128], BF16)
            make_identity(nc, ident[:])
            epst = cpool.tile([128, 1], F32)
            nc.gpsimd.memset(epst[:], LN_EPS)
            wnc_sb = cpool.tile([KN, EMB + KN], BF16)
            nc.sync.dma_start(wnc_sb[:], WNC[:])
            wab_sb = cpool.tile([EMB, 16], BF16)
            nc.sync.dma_start(wab_sb[:], WAB[:])
            ae9_sb = cpool.tile([128, 2, 16], BF16)
            nc.sync.dma_start(ae9_sb[:], AE9[:])
            wec_sb = cpool.tile([KE, EMB + KE], BF16)
            nc.sync.dma_start(wec_sb[:], WEC[:])

            # ------------- node MLP: 49 tiles, batch 4 for stats
            xout = cpool.tile([128, NWIN, EMB], BF16)
            adout = cpool.tile([128, NWIN, 16], F32)
            xt17 = cpool.tile([KN, NPC], BF16)
            nc.sync.dma_start(xt17[:], XT17[:])
            NG = 0 if skip_node else (NWIN + 3) // 4
            with (
                tc.tile_pool(name="na", bufs=3) as na,
                tc.tile_pool(name="na_ps", bufs=2, space="PSUM") as nap,
                tc.tile_pool(name="nt_ps", bufs=2, space="PSUM") as ntp,
                tc.tile_pool(name="nad_ps", bufs=2, space="PSUM") as nadp,
            ):
                for g in range(NG):
                    t0 = 4 * g
                    nt = min(4, NWIN - t0)
                    y4 = nap.tile([128, 4, EMB + KN], F32, tag="y4")
                    for t in range(nt):
                        nc.tensor.matmul(
                            y4[:, t, :],
                            lhsT=xt17[:, 128 * (t0 + t) : 128 * (t0 + t + 1)],
                            rhs=wnc_sb[:], start=True, stop=True)
                    u2 = na.tile([128, 4, KN], F32, tag="u2")
                    nc.scalar.activation(u2[:, :nt, :], y4[:, :nt, EMB:],
                                         AF.Square)
                    q = na.tile([128, 4], F32, tag="q")
                    nc.vector.tensor_reduce(q[:, :nt], u2[:, :nt, :],
                                            axis=AX.X, op=OP.add)
                    iv = na.tile([128, 4], F32, tag="iv")
                    nc.scalar.activation(iv[:, :nt], q[:, :nt], AF.Sqrt,
                                         bias=epst[:])
                    nc.vector.reciprocal(iv[:, :nt], iv[:, :nt])
                    for t in range(nt):
                        # x = max(iv*y'g, 0) directly into the table row
                        nc.vector.tensor_scalar(
                            xout[:, t0 + t, :], y4[:, t, :EMB],
                            iv[:, t : t + 1], 0.0, op0=OP.mult, op1=OP.max)
                        rT_ps = ntp.tile([EMB, 128], BF16, tag="rT")
                        nc.tensor.transpose(out=rT_ps[:],
                                            in_=xout[:, t0 + t, :],
                                            identity=ident[:])
                        rT = na.tile([EMB, 128], BF16, tag="rTs")
                        nc.scalar.activation(rT[:], rT_ps[:], AF.Copy)
                        a_ps = nadp.tile([128, 16], F32, tag="aps")
                        nc.tensor.matmul(a_ps[:], lhsT=rT[:], rhs=wab_sb[:],
                                         start=True, stop=True)
                        nc.vector.tensor_scalar(
                            adout[:, t0 + t, :], a_ps[:], 1.0, None,
                            op0=OP.mult)
            if skip_node:
                nc.gpsimd.memset(xout[:], 0.0)
                nc.gpsimd.memset(adout[:], 0.0)
            nc.sync.dma_start(
                XO[:].rearrange("(t p) c -> p t c", p=128), xout[:])
            nc.sync.dma_start(
                AD[:].rearrange("(t p) c -> p t c", p=128), adout[:])

            # ------------- edge MLP: blocks of 8 chunks (2 groups of 4)
            escb = cpool.tile([128, NCHE, 9], F32)
            NBLK = NCHE // 8 if max_blk is None else max_blk
            if max_blk is not None:
                nc.gpsimd.memset(escb[:], 0.0)
            with (
                tc.tile_pool(name="eld", bufs=3) as eld,
                tc.tile_pool(name="ea", bufs=6) as ea,
                tc.tile_pool(name="eb", bufs=4) as eb,
                tc.tile_pool(name="ea_ps", bufs=2, space="PSUM") as eap,
                tc.tile_pool(name="et_ps", bufs=2, space="PSUM") as etp,
                tc.tile_pool(name="ee_ps", bufs=2, space="PSUM") as eep,
            ):
                et = None
                for blk in range(NBLK):
                    c0 = 8 * blk
                    if blk % 2 == 0:
                        et = eld.tile([KE, 16 * 128], BF16, tag="et")
                        nb = min(16, NCHE - c0)
                        nc.sync.dma_start(
                            et[:, : nb * 128],
                            EAT9[:, c0 * 128 : (c0 + nb) * 128])
                    eo = (blk % 2) * 8 * 128
                    y4s = []
                    q8 = eb.tile([128, 8], F32, tag="q8")
                    for h in range(2):
                        y4 = eap.tile([128, 4, EMB + KE], F32,
                                      tag=f"y4{h}")
                        for c in range(4):
                            off = eo + 128 * (4 * h + c)
                            nc.tensor.matmul(
                                y4[:, c, :],
                                lhsT=et[:, off : off + 128],
                                rhs=wec_sb[:], start=True, stop=True)
                        if stop_at >= 2:
                            u2 = ea.tile([128, 4, KE], F32, tag="u2")
                            nc.scalar.activation(u2[:], y4[:, :, EMB:],
                                                 AF.Square)
                            nc.vector.tensor_reduce(
                                q8[:, 4 * h : 4 * h + 4],
                                u2[:], axis=AX.X, op=OP.add)
                        else:
                            nc.gpsimd.memset(q8[:, 4 * h : 4 * h + 4], 1.0)
                        y4s.append(y4)
                    iv8 = eb.tile([128, 8], F32, tag="iv8")
                    if stop_at >= 3:
                        nc.scalar.activation(iv8[:], q8[:], AF.Sqrt,
                                             bias=epst[:])
                        nc.vector.reciprocal(iv8[:], iv8[:])
                    else:
                        nc.gpsimd.memset(iv8[:], 1.0)
                    for h in range(2):
                        y4 = y4s[h]
                        n4 = ea.tile([128, 4, EMB], BF16, tag=f"n4{h}")
                        if stop_at < 4:
                            nc.gpsimd.memset(n4[:], 1.0)
                        for c in range(4 if stop_at >= 4 else 0):
                            ch = 4 * h + c
                            if (c0 + ch) % 8 < 5:
                                nc.vector.tensor_scalar(
                                    n4[:, c, :], y4[:, c, :EMB],
                                    iv8[:, ch : ch + 1], 0.0,
                                    op0=OP.mult, op1=OP.max)
                            else:
                                nc.scalar.activation(
                                    n4[:, c, :], y4[:, c, :EMB], AF.Relu,
                                    scale=iv8[:, ch : ch + 1])
                        zT_ps = etp.tile([128, 2, 128], BF16, tag="zT")
                        for j in range(2 if stop_at >= 5 else 0):
                            nc.tensor.transpose(
                                out=zT_ps[:, j, :],
                                in_=n4[:, 2 * j : 2 * j + 2, :].rearrange(
                                    "p a b -> p (a b)"),
                                identity=ident[:])
                        zT = ea.tile([128, 2, 128], BF16, tag=f"zTs{h}")
                        if stop_at < 6:
                            nc.gpsimd.memset(zT[:], 0.5)
                        elif h == 0:
                            nc.vector.tensor_scalar(zT[:], zT_ps[:], 1.0,
                                                    None, op0=OP.mult)
                        else:
                            nc.scalar.activation(zT[:], zT_ps[:], AF.Copy)
                        e_ps = eep.tile([128, 4, 16], F32, tag="eps")
                        for c in range(4 if stop_at >= 6 else 0):
                            nc.tensor.matmul(
                                e_ps[:, c, :],
                                lhsT=zT[:, c // 2, :],
                                rhs=ae9_sb[:, c % 2, :],
                                start=True, stop=True)
                        if stop_at >= 7:
                            if (blk + h) % 2 == 0:
                                nc.vector.tensor_scalar(
                                    escb[:, c0 + 4 * h : c0 + 4 * h + 4, :],
                                    e_ps[:, :, 0:9], 1.0, None, op0=OP.mult)
                            else:
                                nc.scalar.activation(
                                    escb[:, c0 + 4 * h : c0 + 4 * h + 4, :],
                                    e_ps[:, :, 0:9], AF.Copy)
            nc.sync.dma_start(ESC9[:, :, :], escb[:])
    nc.compile()
    return nc


# ------------------------------------------------------------------ launch 2


def _build_launch2(KT):
    NS = KT * 128
    nc = bacc.Bacc("TRN2", target_bir_lowering=False, debug=False,
                   num_devices=NCORES)
    din = lambda n, s, d=F32: nc.dram_tensor(n, s, d, kind="ExternalInput")
    XG = din("XG", [NWIN, 128, KT, EMB], BF16)
    CMT = din("CMT", [NWIN, 24, NS], F32)
    S3H = din("S3H", [NWIN, 128, KT, 128], BF16)
    E24 = din("E24", [24, 8], F32)
    OWC = din("OWC", [128, 4, 66], BF16)    # [out_W | oa_src | oa_tgt] blocks
    WB4 = din("WB4", [128, 4, 128], BF16)   # block-diag gat_W head pairs
    HOUTS = nc.dram_tensor("HOUTS", [NPC, 66], F32, kind="ExternalOutput")

    vp = _vpaths(KT)

    with tile.TileContext(nc) as tc:
        with tc.tile_pool(name="const", bufs=1) as cpool:
            ident = cpool.tile([128, 128], BF16)
            make_identity(nc, ident[:])
            negone = cpool.tile([128, 1], F32)
            nc.gpsimd.memset(negone[:], -1.0)
            slp = cpool.tile([128, 1], F32)
            nc.gpsimd.memset(slp[:], SLOPE)
            nslp = cpool.tile([128, 1], F32)
            nc.gpsimd.memset(nslp[:], -SLOPE)
            e24_sb = cpool.tile([24, 8], F32)
            nc.sync.dma_start(e24_sb[:], E24[:])
            owc_sb = cpool.tile([128, 4, 66], BF16)
            nc.sync.dma_start(owc_sb[:], OWC[:])
            wb4_sb = cpool.tile([128, 4, 128], BF16)
            nc.sync.dma_start(wb4_sb[:], WB4[:])
            with (
                tc.tile_pool(name="w", bufs=3) as wp,
                tc.tile_pool(name="wv", bufs=6) as wv,
                tc.tile_pool(name="ws_ps", bufs=2, space="PSUM") as wsp,
                tc.tile_pool(name="wn_ps", bufs=2, space="PSUM") as wnp,
                tc.tile_pool(name="wt_ps", bufs=2, space="PSUM") as wtp,
            ):
                for w in range(NWIN):
                    xg = wp.tile([128, KT, EMB], BF16, tag="xg")
                    nc.sync.dma_start(xg[:], XG[w])
                    cmt = wp.tile([24, NS], F32, tag="cmt")
                    nc.sync.dma_start(cmt[:], CMT[w])
                    s3 = wp.tile([128, KT, 128], BF16, tag="s3")
                    nc.sync.dma_start(s3[:], S3H[w])
                    # scores: s8 = sum of components via PE
                    psu = wsp.tile([128, KT * 8 + 74], F32, tag="s8u")
                    s8_ps = psu[:, : KT * 8].rearrange(
                        "p (c i) -> p c i", i=8)
                    den_ps = psu[:, KT * 8 + 66 : KT * 8 + 74]
                    for c in range(KT):
                        nc.tensor.matmul(s8_ps[:, c, :],
                                         lhsT=cmt[:, 128 * c : 128 * (c + 1)],
                                         rhs=e24_sb[:], start=True, stop=True)
                    # w8 = exp(lrelu(s)) = max(exp(s), exp(0.01 s))
                    ex1 = wv.tile([128, KT, 8], BF16, tag="ex1")
                    nc.scalar.activation(ex1[:], s8_ps, AF.Exp)
                    ex2 = wv.tile([128, KT, 8], BF16, tag="ex2")
                    nc.scalar.activation(ex2[:], s8_ps, AF.Exp,
                                         scale=slp[:])
                    w8 = wv.tile([128, KT, 8], BF16, tag="w8")
                    nc.vector.tensor_tensor(out=w8[:], in0=ex1[:],
                                            in1=ex2[:], op=OP.max)
                    # V per chunk (DVE / Pool split) + one-hot matmuls
                    num_ps = wnp.tile([128, 512], F32, tag="num")
                    for c in range(KT):
                        V = wv.tile([128, HEADS, EMB], BF16, tag="V")
                        if vp[c] == "D":
                            nc.vector.tensor_tensor(
                                out=V[:],
                                in0=xg[:, c : c + 1, :].to_broadcast(
                                    [128, 8, EMB]),
                                in1=w8[:, c, :].to_broadcast([128, 8, EMB]),
                                op=OP.mult)
                        else:
                            nc.gpsimd.tensor_tensor(
                                out=V[:],
                                in0=xg[:, c : c + 1, :].to_broadcast(
                                    [128, 8, EMB]),
                                in1=w8[:, c, :].to_broadcast([128, 8, EMB]),
                                op=OP.mult)
                        nc.tensor.matmul(num_ps[:],
                                         lhsT=s3[:, c, :],
                                         rhs=V[:].rearrange(
                                             "p i f -> p (i f)"),
                                         start=(c == 0), stop=(c == KT - 1))
                        nc.tensor.matmul(den_ps, lhsT=s3[:, c, :],
                                         rhs=w8[:, c, :],
                                         start=(c == 0), stop=(c == KT - 1))
                    den = wv.tile([128, 8], F32, tag="dens")
                    nc.vector.tensor_scalar(den[:], den_ps, 1e-16, None,
                                            op0=OP.add)
                    nc.vector.reciprocal(den[:], den[:])
                    xh = wv.tile([128, 512], BF16, tag="xh")
                    nc.vector.tensor_tensor(
                        out=xh[:].rearrange("p (i f) -> p i f", f=EMB),
                        in0=num_ps[:].rearrange("p (i f) -> p i f", f=EMB),
                        in1=den[:].to_broadcast([128, 8, EMB]), op=OP.mult)
                    ho_ps = psu[:, KT * 8 : KT * 8 + 66]
                    # per-head W: transpose agg, W-matmul (stays f-major)
                    hh = wv.tile([128, 4, 128], BF16, tag="hh")
                    for j in range(4):
                        xT_ps = wtp.tile([128, 128], BF16, tag="xT")
                        nc.tensor.transpose(
                            out=xT_ps[:], in_=xh[:, 128 * j : 128 * (j + 1)],
                            identity=ident[:])
                        xT = wv.tile([128, 128], BF16, tag="xTs")
                        if j % 2 == 0:
                            nc.scalar.activation(xT[:], xT_ps[:], AF.Copy)
                        else:
                            nc.vector.tensor_scalar(xT[:], xT_ps[:], 1.0,
                                                    None, op0=OP.mult)
                        hT_ps = wtp.tile([128, 128], F32, tag="hT")
                        nc.tensor.matmul(hT_ps[:], lhsT=wb4_sb[:, j, :],
                                         rhs=xT[:], start=True, stop=True)
                        if j % 2 == 0:
                            nc.vector.tensor_scalar(hh[:, j, :], hT_ps[:],
                                                    1.0, None, op0=OP.mult)
                        else:
                            nc.scalar.activation(hh[:, j, :], hT_ps[:],
                                                 AF.Copy)
                    # elu(elu(.)) in f-major, batched over the 4 blocks
                    m0 = wv.tile([128, 512], BF16, tag="m0")
                    nc.vector.tensor_scalar_min(
                        m0[:], hh[:].rearrange("p a b -> p (a b)"), 0.0)
                    nc.scalar.activation(m0[:], m0[:], AF.Exp)
                    nc.scalar.activation(m0[:], m0[:], AF.Exp,
                                         bias=negone[:])
                    r0 = wv.tile([128, 512], BF16, tag="r0")
                    nc.vector.tensor_scalar(
                        r0[:], hh[:].rearrange("p a b -> p (a b)"), 0.0,
                        -1.0, op0=OP.max, op1=OP.add)
                    xh2 = wv.tile([128, 4, 128], BF16, tag="xh2")
                    nc.vector.tensor_tensor(
                        out=xh2[:].rearrange("p a b -> p (a b)"), in0=m0[:],
                        in1=r0[:], op=OP.add)
                    # out layer from f-major xh2 blocks
                    for j in range(4):
                        nc.tensor.matmul(ho_ps, lhsT=xh2[:, j, :],
                                         rhs=owc_sb[:, j, :],
                                         start=(j == 0), stop=(j == 3))
                    hrow = wv.tile([128, 66], F32, tag="hrow")
                    nc.scalar.activation(hrow[:], ho_ps, AF.Copy)
                    nc.sync.dma_start(HOUTS[128 * w : 128 * (w + 1), :],
                                      hrow[:])
    nc.compile()
    return nc


# ------------------------------------------------------------------ launch 3


def _build_launch3(KT):
    NS = KT * 128
    nc = bacc.Bacc("TRN2", target_bir_lowering=False, debug=False,
                   num_devices=NCORES)
    din = lambda n, s, d=F32: nc.dram_tensor(n, s, d, kind="ExternalInput")
    HG = din("HG", [NWIN, 128, KT, OUT], BF16)
    CM2 = din("CM2", [NWIN, 4, NS], F32)    # e9 | asrcO | atgtO | 0
    S3D = din("S3D", [NWIN, 128, NDMA3, 128], BF16)   # first NDMA3 chunks
    SRCWF = din("SRCWF", [128, NWIN, KT], F32)
    OUTT = nc.dram_tensor("OUTT", [NPC, OUT], F32, kind="ExternalOutput")

    with tile.TileContext(nc) as tc:
        with tc.tile_pool(name="const", bufs=1) as cpool:
            e4 = cpool.tile([4, 1], F32)
            nc.gpsimd.memset(e4[:], 1.0)
            slp = cpool.tile([128, 1], F32)
            nc.gpsimd.memset(slp[:], SLOPE)
            iota_bf = cpool.tile([128, 128], BF16)
            nc.gpsimd.iota(iota_bf[:], pattern=[[1, 128]], base=0,
                           channel_multiplier=0,
                           allow_small_or_imprecise_dtypes=True)
            srcwf = cpool.tile([128, NWIN, KT], F32)
            nc.sync.dma_start(srcwf[:], SRCWF[:])
            hall = cpool.tile([128, NWIN, OUT], F32)
            with (
                tc.tile_pool(name="w", bufs=3) as wp,
                tc.tile_pool(name="wv", bufs=6) as wv,
                tc.tile_pool(name="ws_ps", bufs=2, space="PSUM") as wsp,
                tc.tile_pool(name="wn_ps", bufs=2, space="PSUM") as wnp,
                tc.tile_pool(name="wd_ps", bufs=2, space="PSUM") as wdp,
            ):
                for w in range(NWIN):
                    hg = wp.tile([128, KT, OUT], BF16, tag="hg")
                    nc.sync.dma_start(hg[:], HG[w])
                    cm2 = wp.tile([4, NS], F32, tag="cm2")
                    nc.sync.dma_start(cm2[:], CM2[w])
                    s3 = wp.tile([128, KT, 128], BF16, tag="s3")
                    nc.sync.dma_start(s3[:, :NDMA3, :], S3D[w])
                    # build remaining one-hot chunks on DVE / Pool
                    for c in range(NDMA3, KT):
                        if c % 2 == 0:
                            nc.vector.tensor_scalar(
                                s3[:, c, :], iota_bf[:],
                                srcwf[:, w, c : c + 1], None,
                                op0=OP.is_equal)
                        else:
                            nc.gpsimd.tensor_scalar(
                                s3[:, c, :], iota_bf[:],
                                srcwf[:, w, c : c + 1], None,
                                op0=OP.is_equal)
                    s1_ps = wsp.tile([128, KT], F32, tag="s1")
                    for c in range(KT):
                        nc.tensor.matmul(s1_ps[:, c : c + 1],
                                         lhsT=cm2[:, 128 * c : 128 * (c + 1)],
                                         rhs=e4[:], start=True, stop=True)
                    # w1 = max(exp(s), exp(0.01 s))
                    ex1 = wv.tile([128, KT], BF16, tag="ex1")
                    nc.scalar.activation(ex1[:], s1_ps[:], AF.Exp)
                    ex2 = wv.tile([128, KT], BF16, tag="ex2")
                    nc.scalar.activation(ex2[:], s1_ps[:], AF.Exp,
                                         scale=slp[:])
                    w1 = wv.tile([128, KT], BF16, tag="w1")
                    nc.vector.tensor_tensor(out=w1[:], in0=ex1[:],
                                            in1=ex2[:], op=OP.max)
                    V1 = wv.tile([128, KT, OUT], BF16, tag="V1")
                    h3 = KT // 3
                    nc.vector.tensor_tensor(
                        out=V1[:, : 2 * h3, :], in0=hg[:, : 2 * h3, :],
                        in1=w1[:, : 2 * h3].to_broadcast(
                            [128, 2 * h3, OUT]), op=OP.mult)
                    nc.gpsimd.tensor_tensor(
                        out=V1[:, 2 * h3 :, :], in0=hg[:, 2 * h3 :, :],
                        in1=w1[:, 2 * h3 :].to_broadcast(
                            [128, KT - 2 * h3, OUT]),
                        op=OP.mult)
                    num_ps = wnp.tile([128, OUT], F32, tag="num")
                    den_ps = wdp.tile([128, 1], F32, tag="den")
                    for c in range(KT):
                        nc.tensor.matmul(num_ps[:], lhsT=s3[:, c, :],
                                         rhs=V1[:, c, :],
                                         start=(c == 0), stop=(c == KT - 1))
                        nc.tensor.matmul(den_ps[:], lhsT=s3[:, c, :],
                                         rhs=w1[:, c : c + 1],
                                         start=(c == 0), stop=(c == KT - 1))
                    den = wv.tile([128, 1], F32, tag="dens")
                    nc.vector.tensor_scalar(den[:], den_ps[:], 1e-16, None,
                                            op0=OP.add)
                    nc.vector.reciprocal(den[:], den[:])
                    h2 = wv.tile([128, OUT], F32, tag="h2")
                    nc.vector.tensor_scalar(h2[:], num_ps[:], den[:], None,
                                            op0=OP.mult)
                    m0 = wv.tile([128, OUT], F32, tag="m0")
                    nc.vector.tensor_scalar_min(m0[:], h2[:], 0.0)
                    nc.scalar.activation(m0[:], m0[:], AF.Exp)
                    r0 = wv.tile([128, OUT], F32, tag="r0")
                    nc.vector.tensor_scalar(r0[:], h2[:], 0.0, -1.0,
                                            op0=OP.max, op1=OP.add)
                    nc.vector.tensor_tensor(out=hall[:, w, :], in0=m0[:],
                                            in1=r0[:], op=OP.add)
            with tc.tile_pool(name="fin", bufs=1) as fin:
                ex = fin.tile([128, NWIN, OUT], F32)
                nc.scalar.activation(ex[:], hall[:], AF.Exp)
                sm = fin.tile([128, NWIN], F32)
                nc.vector.tensor_reduce(sm[:], ex[:], axis=AX.X, op=OP.add)
                nc.scalar.activation(sm[:], sm[:], AF.Ln)
                res = fin.tile([128, NWIN, OUT], F32)
                nc.vector.tensor_tensor(
                    out=res[:], in0=hall[:],
                    in1=sm[:].to_broadcast([128, NWIN, OUT]), op=OP.subtract)
                nc.sync.dma_start(
                    OUTT[:].rearrange("(w p) f -> p w f", p=128), res[:])
    nc.compile()
    return nc


# ------------------------------------------------------------------ driver


def kernel(X, edge_attr, w_node, b_node, g_node, beta_node,
           w_edge, b_edge, g_edge, beta_edge,
           gat_W, gat_a, out_W, out_a,
           edge_index, matched_car_infra_nodes):
    import ml_dtypes
    import time as _time

    bf = lambda a: np.ascontiguousarray(np.asarray(a, np.float32)).astype(
        ml_dtypes.bfloat16)
    f32 = lambda a: np.ascontiguousarray(np.asarray(a, np.float32))

    X = f32(X)
    ea = f32(edge_attr)
    w_node = f32(w_node); b_node = f32(b_node); g_node = f32(g_node)
    beta_node = f32(beta_node)
    w_edge = f32(w_edge); b_edge = f32(b_edge); g_edge = f32(g_edge)
    beta_edge = f32(beta_edge)
    gW = f32(gat_W); ga = f32(gat_a); oW = f32(out_W); oa = f32(out_a)
    assert np.abs(beta_node).max() < 1e-6 and np.abs(beta_edge).max() < 1e-6

    per_core, pnode, origin, KT, srcw_of, ptgt = _prep(edge_index)
    NS = KT * 128
    NCHE = NWIN * KT
    NCHE4 = ((NCHE + 15) // 16) * 16

    # ---- LN-folded weights (centered + Cholesky u-columns)
    def fold(Wb, bb, g, kdim):
        Wfull = np.concatenate([Wb, bb[None, :]], 0)          # [k, 64]
        m = Wfull.mean(axis=1)                                 # [k]
        Wc = Wfull - m[:, None]
        M = Wc @ Wc.T + 1e-10 * np.eye(kdim)
        B = np.linalg.cholesky(M) / np.sqrt(EMB)
        return np.concatenate([Wc * g[None, :], B], 1)         # [k, 64+k]

    WNC = bf(fold(w_node, b_node, g_node, NODE_DIM + 1))
    WEC = bf(fold(w_edge, b_edge, g_edge, EA_DIM + 1))
    WAB = np.zeros((EMB, 16), np.float32)
    for i in range(HEADS):
        WAB[:, i] = gW[i] @ ga[i, :OUT]
        WAB[:, 8 + i] = gW[i] @ ga[i, OUT : 2 * OUT]
    WAB = bf(WAB)
    AE9 = np.zeros((EMB, 16), np.float32)
    for i in range(HEADS):
        AE9[:, i] = ga[i, 2 * OUT :]
    AE9[:, 8] = oa[2 * OUT :]
    A2 = np.zeros((2, 128, 16), np.float32)
    A2[0, :EMB] = AE9
    A2[1, EMB:] = AE9
    AE9 = bf(A2.transpose(1, 0, 2))

    # ---- launch 1 inputs
    Xp = np.zeros((NPN, NODE_DIM + 1), np.float32)
    valid = origin >= 0
    Xp[valid, :NODE_DIM] = X[origin[valid]]
    Xp[:, NODE_DIM] = 1.0
    src = np.asarray(edge_index[0]).astype(np.int64)

    in_maps1 = []
    for k in range(NCORES):
        eslot = per_core[k]                                    # [NWIN, NS]
        eat = np.zeros((NCHE4 * 128, EA_DIM + 1), np.float32)
        es = eslot.reshape(-1)
        m = es >= 0
        eat[: NS * NWIN][m, :EA_DIM] = ea[es[m]]
        eat[: NS * NWIN][m, EA_DIM] = 1.0
        in_maps1.append(dict(
            XT17=bf(Xp[k * NPC : (k + 1) * NPC].T),
            WNC=WNC, WAB=WAB, AE9=AE9,
            EAT9=bf(eat.T), WEC=WEC))

    nc1 = _build_launch1(NCHE4)
    kernel.nc1 = nc1
    _t = _time.perf_counter()
    res1 = run_bass_kernel_spmd(nc1, in_maps1, core_ids=list(range(NCORES)))
    kernel.wall1 = _time.perf_counter() - _t

    # ---- host: assemble tables, gather per-slot inputs for launch 2
    XF = np.zeros((NPN, EMB), ml_dtypes.bfloat16)
    ADF = np.zeros((NPN, 16), np.float32)
    ESCF = []
    for k in range(NCORES):
        XF[k * NPC : (k + 1) * NPC] = res1.results[k]["XO"]
        ADF[k * NPC : (k + 1) * NPC] = res1.results[k]["AD"]
        # ESC9 [128, NCHE4, 9] -> slot-major [NWIN, NS, 9]
        e9 = np.asarray(res1.results[k]["ESC9"], np.float32)[:, :NCHE, :]
        e9 = e9.transpose(1, 0, 2).reshape(NWIN, NS, 9)
        ESCF.append(e9)

    # one-hot S3 per core (shared by launches 2 and 3)
    in_maps2 = []
    s3_cores = []
    for k in range(NCORES):
        eslot = per_core[k]
        es = eslot.reshape(NWIN, NS)
        m = es >= 0
        tgtrow = np.zeros((NWIN, NS), np.int64)
        tgtrow[m] = ptgt[es[m]]
        srcw = np.full((NWIN, NS), -1, np.int64)
        srcw[m] = srcw_of[es[m]]

        XGk = np.zeros((NWIN, NS, EMB), ml_dtypes.bfloat16)
        XGk[m] = XF[tgtrow[m]]
        CMTk = np.zeros((NWIN, 24, NS), np.float32)
        CMTk[:, 0:8, :] = ESCF[k][:, :, 0:8].transpose(0, 2, 1)
        srcrow_k = np.zeros((NWIN, NS), np.int64)
        # src row = core base + win*128 + srcw
        wid = np.arange(NWIN)[:, None]
        srcrow_k[m] = (k * NPC + (wid + np.zeros_like(srcw))[m] * 128
                       + srcw[m])
        asrc = np.zeros((NWIN, NS, 8), np.float32)
        asrc[m] = ADF[srcrow_k[m], 0:8]
        atgt = np.zeros((NWIN, NS, 8), np.float32)
        atgt[m] = ADF[tgtrow[m], 8:16]
        CMTk[:, 8:16, :] = asrc.transpose(0, 2, 1)
        CMTk[:, 16:24, :] = atgt.transpose(0, 2, 1)

        S3k = np.zeros((NWIN, NS, 128), ml_dtypes.bfloat16)
        ww, ss = np.nonzero(m)
        S3k[ww, ss, srcw[ww, ss]] = 1.0
        S3k = S3k.reshape(NWIN, KT, 128, 128).transpose(0, 2, 1, 3)
        s3_cores.append(np.ascontiguousarray(S3k))

        WB4 = np.zeros((128, 4, 128), np.float32)
        for j in range(4):
            for il in range(2):
                WB4[64 * il : 64 * il + 64, j,
                    64 * il : 64 * il + 64] = gW[2 * j + il]
        E24 = np.zeros((24, 8), np.float32)
        for i in range(8):
            E24[i, i] = 1.0
            E24[8 + i, i] = 1.0
            E24[16 + i, i] = 1.0
        OWC = np.zeros((512, 66), np.float32)
        OWC[:, 0:64] = oW
        OWC[:, 64] = oW @ oa[:OUT]
        OWC[:, 65] = oW @ oa[OUT : 2 * OUT]
        in_maps2.append(dict(
            XG=_slotmaj(XGk, KT, EMB),
            CMT=CMTk,
            S3H=s3_cores[k],
            E24=E24,
            OWC=bf(np.ascontiguousarray(
                OWC.reshape(4, 128, 66).transpose(1, 0, 2))),
            WB4=bf(WB4),
        ))

    nc2 = _build_launch2(KT)
    kernel.nc2 = nc2
    _t = _time.perf_counter()
    res2 = run_bass_kernel_spmd(nc2, in_maps2, core_ids=list(range(NCORES)))
    kernel.wall2 = _time.perf_counter() - _t

    # ---- host: assemble h_out table, gather for launch 3
    HF = np.zeros((NPN, 66), np.float32)
    for k in range(NCORES):
        HF[k * NPC : (k + 1) * NPC] = res2.results[k]["HOUTS"]
    HFb = HF[:, 0:64].astype(ml_dtypes.bfloat16)

    in_maps3 = []
    for k in range(NCORES):
        eslot = per_core[k]
        es = eslot.reshape(NWIN, NS)
        m = es >= 0
        tgtrow = np.zeros((NWIN, NS), np.int64)
        tgtrow[m] = ptgt[es[m]]
        srcw = np.full((NWIN, NS), -1, np.int64)
        srcw[m] = srcw_of[es[m]]
        wid = np.arange(NWIN)[:, None]
        srcrow_k = np.zeros((NWIN, NS), np.int64)
        srcrow_k[m] = (k * NPC + (wid + np.zeros_like(srcw))[m] * 128
                       + srcw[m])

        HGk = np.zeros((NWIN, NS, OUT), ml_dtypes.bfloat16)
        HGk[m] = HFb[tgtrow[m]]
        CM2k = np.zeros((NWIN, 4, NS), np.float32)
        CM2k[:, 0, :] = ESCF[k][:, :, 8]
        a_s = np.zeros((NWIN, NS), np.float32)
        a_s[m] = HF[srcrow_k[m], 64]
        a_t = np.zeros((NWIN, NS), np.float32)
        a_t[m] = HF[tgtrow[m], 65]
        CM2k[:, 1, :] = a_s
        CM2k[:, 2, :] = a_t
        srcwf_f = srcw.reshape(NWIN, KT, 128).transpose(2, 0, 1).astype(
            np.float32)
        in_maps3.append(dict(
            HG=_slotmaj(HGk, KT, OUT),
            CM2=CM2k,
            S3D=np.ascontiguousarray(s3_cores[k][:, :, :NDMA3, :]),
            SRCWF=np.ascontiguousarray(srcwf_f)))

    nc3 = _build_launch3(KT)
    kernel.nc3 = nc3
    _t = _time.perf_counter()
    res3 = run_bass_kernel_spmd(nc3, in_maps3, core_ids=list(range(NCORES)))
    kernel.wall3 = _time.perf_counter() - _t

    outp = np.zeros((NPN, OUT), np.float32)
    for k in range(NCORES):
        outp[k * NPC : (k + 1) * NPC] = res3.results[k]["OUTT"]
    out = np.zeros((N, OUT), np.float32)
    valid = origin >= 0
    out[origin[valid]] = outp[valid]
    return out


def _slotmaj(A, KT, F):
    """[NWIN, NS, F] with slot s=(c*128+p) -> [NWIN, 128, KT, F]."""
    NW = A.shape[0]
    return np.ascontiguousarray(
        A.reshape(NW, KT, 128, F).transpose(0, 2, 1, 3))


# revision 48
# speedup vs baseline: 2.2584x; 1.0047x over previous
"""AA_GAT on 8 trn2 cores (self-contained), v2.

Three launches; host does layout/gather only between launches.

L1: node MLP (nodes sharded 1/8 per core) + edge MLP (edges sharded by
    src-window). LN via Cholesky trick: y' = centered pre-LN output and
    u-columns come out of one matmul; var = sum(u^2)/64. beta=0 lets
    relu commute with the 1/sigma scale, so the only PSUM->SBUF bridge
    is a plain Relu; the iv scale is applied to the tiny outputs
    (adots 16 cols, esc 9 cols, x 64 cols once per node tile).
L2: layer-1 8-head edge pass per src window. Scores summed on PE from
    a host-transposed component table (esc8|asrc8|atgt8); exp on Act;
    per-edge value weighting V = w8 (x) xg via three engine paths
    (Act-replicate + DVE-stt / DVE tensor_tensor / Pool stt), one-hot
    segment-sum matmuls (host-prebuilt S3), elu(elu(.)), out-layer
    h_out = xh @ out_W + alpha dots.
L3: out-layer edge pass (same slot layout), then batched log_softmax.
"""

import numpy as np

import concourse.bass as bass
import concourse.mybir as mybir
import concourse.tile as tile
from concourse import bacc
from concourse.bass_utils import run_bass_kernel_spmd
from concourse.masks import make_identity

F32 = mybir.dt.float32
BF16 = mybir.dt.bfloat16
AF = mybir.ActivationFunctionType
OP = mybir.AluOpType
AX = mybir.AxisListType

N = 50000
E = 1_000_000
NODE_DIM = 16
EMB = 64
OUT = 64
HEADS = 8
EA_DIM = 8
SLOPE = 0.01
LN_EPS = 1e-5

NCORES = 8
NWIN = 49                 # windows (128 src nodes) per core
NPC = NWIN * 128          # 6272 nodes per core
NPN = NCORES * NPC        # 50176 padded node count
NWTOT = NCORES * NWIN     # 392 windows total
NDMA3 = 8                 # launch-3 one-hot chunks loaded via DMA

# L2 per-chunk V-path assignment (tuned): 'B' Act-replicate + DVE stt,
# 'A' DVE tensor_tensor broadcast, 'C' Pool stt broadcast.


def _vpaths(KT):
    # A = Act double-replicate + packed DVE TT (2x), D = DVE TT broadcast,
    # P = Pool TT broadcast; 2:11:7 split
    order = "DPDPADDPDPDPDPADDPDP" * 4
    return [order[c % len(order)] for c in range(KT)]


# ------------------------------------------------------------------ host prep


def _prep(edge_index):
    """Degree-balanced node->window permutation and edge slot layout."""
    src = np.asarray(edge_index[0]).astype(np.int64)
    tgt = np.asarray(edge_index[1]).astype(np.int64)

    deg = np.bincount(src, minlength=N).astype(np.int64)
    # greedy: big-degree nodes first, into least-loaded window with space
    order = np.argsort(-deg, kind="stable")
    wload = np.zeros(NWTOT, np.int64)
    wcnt = np.zeros(NWTOT, np.int64)
    wnodes = [[] for _ in range(NWTOT)]
    import heapq

    heap = [(0, 0, w) for w in range(NWTOT)]
    heapq.heapify(heap)
    for n in order:
        while True:
            load, cnt, w = heapq.heappop(heap)
            if wcnt[w] < 128:
                break
        wnodes[w].append(n)
        wload[w] += deg[n]
        wcnt[w] += 1
        if wcnt[w] < 128:
            heapq.heappush(heap, (wload[w], wcnt[w], w))
    # order windows by load, snake-assign to cores for balance
    worder = np.argsort(-wload, kind="stable")
    core_wins = [[] for _ in range(NCORES)]
    fwd = True
    i = 0
    while i < NWTOT:
        rng = range(NCORES) if fwd else range(NCORES - 1, -1, -1)
        for k in rng:
            if i < NWTOT:
                core_wins[k].append(worder[i])
                i += 1
        fwd = not fwd
    # global permuted row id: core k, local window j, slot s
    pnode = np.full(N, -1, np.int64)
    origin = np.full(NPN, -1, np.int64)
    for k in range(NCORES):
        for j, w in enumerate(core_wins[k]):
            base = k * NPC + j * 128
            nodes = wnodes[w]
            for s, n in enumerate(nodes):
                pnode[n] = base + s
                origin[base + s] = n
    assert (pnode >= 0).all()

    psrc = pnode[src]
    ptgt = pnode[tgt]
    core_of = psrc // NPC
    win_of = (psrc % NPC) // 128
    srcw_of = psrc % 128

    KT = 0
    buckets = {}
    for k in range(NCORES):
        mk = core_of == k
        idx_k = np.nonzero(mk)[0]
        w = win_of[idx_k]
        for ww in range(NWIN):
            el = idx_k[w == ww]
            buckets[(k, ww)] = el
            KT = max(KT, (len(el) + 127) // 128)
    NS = KT * 128          # slots per window

    per_core = []
    for k in range(NCORES):
        eslot = np.full((NWIN, NS), -1, np.int64)     # edge id per slot
        for ww in range(NWIN):
            el = buckets[(k, ww)]
            eslot[ww, : len(el)] = el
        per_core.append(eslot)
    return per_core, pnode, origin, KT, srcw_of, ptgt


# ------------------------------------------------------------------ launch 1


def _build_launch1(NCHE, skip_node=False, max_blk=None, stop_at=99):
    """Node MLP (49 tiles) + edge MLP (NCHE chunks)."""
    nc = bacc.Bacc("TRN2", target_bir_lowering=False, debug=False,
                   num_devices=NCORES)
    din = lambda n, s, d=F32: nc.dram_tensor(n, s, d, kind="ExternalInput")
    XT17 = din("XT17", [NODE_DIM + 1, NPC], BF16)
    WNC = din("WNC", [NODE_DIM + 1, EMB + NODE_DIM + 1], BF16)
    WAB = din("WAB", [EMB, 16], BF16)
    AE9 = din("AE9", [128, 2, 16], BF16)     # [AE9;0] and [0;AE9] halves
    EAT9 = din("EAT9", [EA_DIM + 1, NCHE * 128], BF16)
    WEC = din("WEC", [EA_DIM + 1, EMB + EA_DIM + 1], BF16)

    XO = nc.dram_tensor("XO", [NPC, EMB], BF16, kind="ExternalOutput")
    AD = nc.dram_tensor("AD", [NPC, 16], F32, kind="ExternalOutput")
    ESC9 = nc.dram_tensor("ESC9", [128, NCHE, 9], F32, kind="ExternalOutput")

    KN = NODE_DIM + 1   # 17 u-cols (node)
    KE = EA_DIM + 1     # 9 u-cols (edge)

    with tile.TileContext(nc) as tc:
        with tc.tile_pool(name="const", bufs=1) as cpool:
            ident = cpool.tile([128, 128], BF16)
            make_identity(nc, ident[:])
            epst = cpool.tile([128, 1], F32)
            nc.gpsimd.memset(epst[:], LN_EPS)
            wnc_sb = cpool.tile([KN, EMB + KN], BF16)
            nc.sync.dma_start(wnc_sb[:], WNC[:])
            wab_sb = cpool.tile([EMB, 16], BF16)
            nc.sync.dma_start(wab_sb[:], WAB[:])
            ae9_sb = cpool.tile([128, 2, 16], BF16)
            nc.sync.dma_start(ae9_sb[:], AE9[:])
            wec_sb = cpool.tile([KE, EMB + KE], BF16)
            nc.sync.dma_start(wec_sb[:], WEC[:])

            # ------------- node MLP: 49 tiles, batch 4 for stats
            xout = cpool.tile([128, NWIN, EMB], BF16)
            adout = cpool.tile([128, NWIN, 16], F32)
            xt17 = cpool.tile([KN, NPC], BF16)
            nc.sync.dma_start(xt17[:], XT17[:])
            NG = 0 if skip_node else (NWIN + 3) // 4
            with (
                tc.tile_pool(name="na", bufs=3) as na,
                tc.tile_pool(name="na_ps", bufs=2, space="PSUM") as nap,
                tc.tile_pool(name="nt_ps", bufs=2, space="PSUM") as ntp,
                tc.tile_pool(name="nad_ps", bufs=2, space="PSUM") as nadp,
            ):
                for g in range(NG):
                    t0 = 4 * g
                    nt = min(4, NWIN - t0)
                    y4 = nap.tile([128, 4, EMB + KN], F32, tag="y4")
                    for t in range(nt):
                        nc.tensor.matmul(
                            y4[:, t, :],
                            lhsT=xt17[:, 128 * (t0 + t) : 128 * (t0 + t + 1)],
                            rhs=wnc_sb[:], start=True, stop=True)
                    u2 = na.tile([128, 4, KN], F32, tag="u2")
                    nc.scalar.activation(u2[:, :nt, :], y4[:, :nt, EMB:],
                                         AF.Square)
                    q = na.tile([128, 4], F32, tag="q")
                    nc.vector.tensor_reduce(q[:, :nt], u2[:, :nt, :],
                                            axis=AX.X, op=OP.add)
                    iv = na.tile([128, 4], F32, tag="iv")
                    nc.scalar.activation(iv[:, :nt], q[:, :nt], AF.Sqrt,
                                         bias=epst[:])
                    nc.vector.reciprocal(iv[:, :nt], iv[:, :nt])
                    for t in range(nt):
                        # x = max(iv*y'g, 0) directly into the table row
                        nc.vector.tensor_scalar(
                            xout[:, t0 + t, :], y4[:, t, :EMB],
                            iv[:, t : t + 1], 0.0, op0=OP.mult, op1=OP.max)
                        rT_ps = ntp.tile([EMB, 128], BF16, tag="rT")
                        nc.tensor.transpose(out=rT_ps[:],
                                            in_=xout[:, t0 + t, :],
                                            identity=ident[:])
                        rT = na.tile([EMB, 128], BF16, tag="rTs")
                        nc.scalar.activation(rT[:], rT_ps[:], AF.Copy)
                        a_ps = nadp.tile([128, 16], F32, tag="aps")
                        nc.tensor.matmul(a_ps[:], lhsT=rT[:], rhs=wab_sb[:],
                                         start=True, stop=True)
                        nc.vector.tensor_scalar(
                            adout[:, t0 + t, :], a_ps[:], 1.0, None,
                            op0=OP.mult)
            if skip_node:
                nc.gpsimd.memset(xout[:], 0.0)
                nc.gpsimd.memset(adout[:], 0.0)
            nc.sync.dma_start(
                XO[:].rearrange("(t p) c -> p t c", p=128), xout[:])
            nc.sync.dma_start(
                AD[:].rearrange("(t p) c -> p t c", p=128), adout[:])

            # ------------- edge MLP: blocks of 8 chunks (2 groups of 4)
            escb = cpool.tile([128, NCHE, 9], F32)
            NBLK = NCHE // 8 if max_blk is None else max_blk
            if max_blk is not None:
                nc.gpsimd.memset(escb[:], 0.0)
            with (
                tc.tile_pool(name="eld", bufs=3) as eld,
                tc.tile_pool(name="ea", bufs=6) as ea,
                tc.tile_pool(name="eb", bufs=4) as eb,
                tc.tile_pool(name="ea_ps", bufs=2, space="PSUM") as eap,
                tc.tile_pool(name="et_ps", bufs=2, space="PSUM") as etp,
                tc.tile_pool(name="ee_ps", bufs=2, space="PSUM") as eep,
            ):
                et = None
                for blk in range(NBLK):
                    c0 = 8 * blk
                    if blk % 2 == 0:
                        et = eld.tile([KE, 16 * 128], BF16, tag="et")
                        nb = min(16, NCHE - c0)
                        nc.sync.dma_start(
                            et[:, : nb * 128],
                            EAT9[:, c0 * 128 : (c0 + nb) * 128])
                    eo = (blk % 2) * 8 * 128
                    y4s = []
                    q8 = eb.tile([128, 8], F32, tag="q8")
                    for h in range(2):
                        y4 = eap.tile([128, 4, EMB + KE], F32,
                                      tag=f"y4{h}")
                        for c in range(4):
                            off = eo + 128 * (4 * h + c)
                            nc.tensor.matmul(
                                y4[:, c, :],
                                lhsT=et[:, off : off + 128],
                                rhs=wec_sb[:], start=True, stop=True)
                        if stop_at >= 2:
                            u2 = ea.tile([128, 4, KE], F32, tag="u2")
                            nc.scalar.activation(u2[:], y4[:, :, EMB:],
                                                 AF.Square)
                            nc.vector.tensor_reduce(
                                q8[:, 4 * h : 4 * h + 4],
                                u2[:], axis=AX.X, op=OP.add)
                        else:
                            nc.gpsimd.memset(q8[:, 4 * h : 4 * h + 4], 1.0)
                        y4s.append(y4)
                    iv8 = eb.tile([128, 8], F32, tag="iv8")
                    if stop_at >= 3:
                        nc.scalar.activation(iv8[:], q8[:], AF.Sqrt,
                                             bias=epst[:])
                        nc.vector.reciprocal(iv8[:], iv8[:])
                    else:
                        nc.gpsimd.memset(iv8[:], 1.0)
                    for h in range(2):
                        y4 = y4s[h]
                        n4 = ea.tile([128, 4, EMB], BF16, tag=f"n4{h}")
                        if stop_at < 4:
                            nc.gpsimd.memset(n4[:], 1.0)
                        for c in range(4 if stop_at >= 4 else 0):
                            ch = 4 * h + c
                            if (c0 + ch) % 8 < 5:
                                nc.vector.tensor_scalar(
                                    n4[:, c, :], y4[:, c, :EMB],
                                    iv8[:, ch : ch + 1], 0.0,
                                    op0=OP.mult, op1=OP.max)
                            else:
                                nc.scalar.activation(
                                    n4[:, c, :], y4[:, c, :EMB], AF.Relu,
                                    scale=iv8[:, ch : ch + 1])
                        zT_ps = etp.tile([128, 2, 128], BF16, tag="zT")
                        for j in range(2 if stop_at >= 5 else 0):
                            nc.tensor.transpose(
                                out=zT_ps[:, j, :],
                                in_=n4[:, 2 * j : 2 * j + 2, :].rearrange(
                                    "p a b -> p (a b)"),
                                identity=ident[:])
                        zT = ea.tile([128, 2, 128], BF16, tag=f"zTs{h}")
                        if stop_at < 6:
                            nc.gpsimd.memset(zT[:], 0.5)
                        elif h == 0:
                            nc.vector.tensor_scalar(zT[:], zT_ps[:], 1.0,
                                                    None, op0=OP.mult)
                        else:
                            nc.scalar.activation(zT[:], zT_ps[:], AF.Copy)
                        e_ps = eep.tile([128, 4, 16], F32, tag="eps")
                        for c in range(4 if stop_at >= 6 else 0):
                            nc.tensor.matmul(
                                e_ps[:, c, :],
                                lhsT=zT[:, c // 2, :],
                                rhs=ae9_sb[:, c % 2, :],
                                start=True, stop=True)
                        if stop_at >= 7:
                            if (blk + h) % 2 == 0:
                                nc.vector.tensor_scalar(
                                    escb[:, c0 + 4 * h : c0 + 4 * h + 4, :],
                                    e_ps[:, :, 0:9], 1.0, None, op0=OP.mult)
                            else:
                                nc.scalar.activation(
                                    escb[:, c0 + 4 * h : c0 + 4 * h + 4, :],
                                    e_ps[:, :, 0:9], AF.Copy)
            nc.sync.dma_start(ESC9[:, :, :], escb[:])
    nc.compile()
    return nc


# ------------------------------------------------------------------ launch 2


def _build_launch2(KT):
    NS = KT * 128
    nc = bacc.Bacc("TRN2", target_bir_lowering=False, debug=False,
                   num_devices=NCORES)
    din = lambda n, s, d=F32: nc.dram_tensor(n, s, d, kind="ExternalInput")
    XG = din("XG", [NWIN, 128, KT, EMB], BF16)
    CMT = din("CMT", [NWIN, 24, NS], F32)
    S3H = din("S3H", [NWIN, 128, KT, 128], BF16)
    E24 = din("E24", [24, 8], F32)
    OWC = din("OWC", [128, 4, 66], BF16)    # [out_W | oa_src | oa_tgt] blocks
    WB4 = din("WB4", [128, 4, 128], BF16)   # block-diag gat_W head pairs
    HOUTS = nc.dram_tensor("HOUTS", [NPC, 66], F32, kind="ExternalOutput")

    vp = _vpaths(KT)

    with tile.TileContext(nc) as tc:
        with tc.tile_pool(name="const", bufs=1) as cpool:
            ident = cpool.tile([128, 128], BF16)
            make_identity(nc, ident[:])
            negone = cpool.tile([128, 1], F32)
            nc.gpsimd.memset(negone[:], -1.0)
            slp = cpool.tile([128, 1], F32)
            nc.gpsimd.memset(slp[:], SLOPE)
            nslp = cpool.tile([128, 1], F32)
            nc.gpsimd.memset(nslp[:], -SLOPE)
            e24_sb = cpool.tile([24, 8], F32)
            nc.sync.dma_start(e24_sb[:], E24[:])
            owc_sb = cpool.tile([128, 4, 66], BF16)
            nc.sync.dma_start(owc_sb[:], OWC[:])
            wb4_sb = cpool.tile([128, 4, 128], BF16)
            nc.sync.dma_start(wb4_sb[:], WB4[:])
            with (
                tc.tile_pool(name="w", bufs=3) as wp,
                tc.tile_pool(name="wv", bufs=6) as wv,
                tc.tile_pool(name="ws_ps", bufs=2, space="PSUM") as wsp,
                tc.tile_pool(name="wn_ps", bufs=2, space="PSUM") as wnp,
                tc.tile_pool(name="wt_ps", bufs=2, space="PSUM") as wtp,
            ):
                for w in range(NWIN):
                    xg = wp.tile([128, KT, EMB], BF16, tag="xg")
                    nc.sync.dma_start(xg[:], XG[w])
                    cmt = wp.tile([24, NS], F32, tag="cmt")
                    nc.sync.dma_start(cmt[:], CMT[w])
                    s3 = wp.tile([128, KT, 128], BF16, tag="s3")
                    nc.sync.dma_start(s3[:], S3H[w])
                    # scores: s8 = sum of components via PE
                    psu = wsp.tile([128, KT * 8 + 74], F32, tag="s8u")
                    s8_ps = psu[:, : KT * 8].rearrange(
                        "p (c i) -> p c i", i=8)
                    den_ps = psu[:, KT * 8 + 66 : KT * 8 + 74]
                    for c in range(KT):
                        nc.tensor.matmul(s8_ps[:, c, :],
                                         lhsT=cmt[:, 128 * c : 128 * (c + 1)],
                                         rhs=e24_sb[:], start=True, stop=True)
                    # w8 = exp(lrelu(s)) = max(exp(s), exp(0.01 s))
                    ex1 = wv.tile([128, KT, 8], BF16, tag="ex1")
                    nc.scalar.activation(ex1[:], s8_ps, AF.Exp)
                    ex2 = wv.tile([128, KT, 8], BF16, tag="ex2")
                    nc.scalar.activation(ex2[:], s8_ps, AF.Exp,
                                         scale=slp[:])
                    w8 = wv.tile([128, KT, 8], BF16, tag="w8")
                    nc.vector.tensor_tensor(out=w8[:], in0=ex1[:],
                                            in1=ex2[:], op=OP.max)
                    # V per chunk (DVE / Pool split) + one-hot matmuls
                    num_ps = wnp.tile([128, 512], F32, tag="num")
                    for c in range(KT):
                        V = wv.tile([128, HEADS, EMB], BF16, tag="V")
                        if vp[c] == "A":
                            w8r = wv.tile([128, HEADS, EMB], BF16,
                                          tag="w8rep")
                            nc.scalar.activation(
                                w8r[:],
                                w8[:, c, :].to_broadcast([128, 8, EMB]),
                                AF.Copy)
                            xgr = wv.tile([128, HEADS, EMB], BF16,
                                          tag="xgrep")
                            nc.scalar.activation(
                                xgr[:],
                                xg[:, c : c + 1, :].to_broadcast(
                                    [128, 8, EMB]),
                                AF.Copy)
                            nc.vector.tensor_tensor(out=V[:], in0=w8r[:],
                                                    in1=xgr[:], op=OP.mult)
                        elif vp[c] == "D":
                            nc.vector.tensor_tensor(
                                out=V[:],
                                in0=xg[:, c : c + 1, :].to_broadcast(
                                    [128, 8, EMB]),
                                in1=w8[:, c, :].to_broadcast([128, 8, EMB]),
                                op=OP.mult)
                        else:
                            nc.gpsimd.tensor_tensor(
                                out=V[:],
                                in0=xg[:, c : c + 1, :].to_broadcast(
                                    [128, 8, EMB]),
                                in1=w8[:, c, :].to_broadcast([128, 8, EMB]),
                                op=OP.mult)
                        nc.tensor.matmul(num_ps[:],
                                         lhsT=s3[:, c, :],
                                         rhs=V[:].rearrange(
                                             "p i f -> p (i f)"),
                                         start=(c == 0), stop=(c == KT - 1))
                        nc.tensor.matmul(den_ps, lhsT=s3[:, c, :],
                                         rhs=w8[:, c, :],
                                         start=(c == 0), stop=(c == KT - 1))
                    den = wv.tile([128, 8], F32, tag="dens")
                    nc.vector.tensor_scalar(den[:], den_ps, 1e-16, None,
                                            op0=OP.add)
                    nc.vector.reciprocal(den[:], den[:])
                    xh = wv.tile([128, 512], BF16, tag="xh")
                    nc.vector.tensor_tensor(
                        out=xh[:].rearrange("p (i f) -> p i f", f=EMB),
                        in0=num_ps[:].rearrange("p (i f) -> p i f", f=EMB),
                        in1=den[:].to_broadcast([128, 8, EMB]), op=OP.mult)
                    ho_ps = psu[:, KT * 8 : KT * 8 + 66]
                    # per-head W: transpose agg, W-matmul (stays f-major)
                    hh = wv.tile([128, 4, 128], BF16, tag="hh")
                    for j in range(4):
                        xT_ps = wtp.tile([128, 128], BF16, tag="xT")
                        nc.tensor.transpose(
                            out=xT_ps[:], in_=xh[:, 128 * j : 128 * (j + 1)],
                            identity=ident[:])
                        xT = wv.tile([128, 128], BF16, tag="xTs")
                        if j % 2 == 0:
                            nc.scalar.activation(xT[:], xT_ps[:], AF.Copy)
                        else:
                            nc.vector.tensor_scalar(xT[:], xT_ps[:], 1.0,
                                                    None, op0=OP.mult)
                        hT_ps = wtp.tile([128, 128], F32, tag="hT")
                        nc.tensor.matmul(hT_ps[:], lhsT=wb4_sb[:, j, :],
                                         rhs=xT[:], start=True, stop=True)
                        if j % 2 == 0:
                            nc.vector.tensor_scalar(hh[:, j, :], hT_ps[:],
                                                    1.0, None, op0=OP.mult)
                        else:
                            nc.scalar.activation(hh[:, j, :], hT_ps[:],
                                                 AF.Copy)
                    # elu(elu(.)) in f-major, batched over the 4 blocks
                    m0 = wv.tile([128, 512], BF16, tag="m0")
                    nc.vector.tensor_scalar_min(
                        m0[:], hh[:].rearrange("p a b -> p (a b)"), 0.0)
                    nc.scalar.activation(m0[:], m0[:], AF.Exp)
                    nc.scalar.activation(m0[:], m0[:], AF.Exp,
                                         bias=negone[:])
                    r0 = wv.tile([128, 512], BF16, tag="r0")
                    nc.vector.tensor_scalar(
                        r0[:], hh[:].rearrange("p a b -> p (a b)"), 0.0,
                        -1.0, op0=OP.max, op1=OP.add)
                    xh2 = wv.tile([128, 4, 128], BF16, tag="xh2")
                    nc.vector.tensor_tensor(
                        out=xh2[:].rearrange("p a b -> p (a b)"), in0=m0[:],
                        in1=r0[:], op=OP.add)
                    # out layer from f-major xh2 blocks
                    for j in range(4):
                        nc.tensor.matmul(ho_ps, lhsT=xh2[:, j, :],
                                         rhs=owc_sb[:, j, :],
                                         start=(j == 0), stop=(j == 3))
                    hrow = wv.tile([128, 66], F32, tag="hrow")
                    nc.scalar.activation(hrow[:], ho_ps, AF.Copy)
                    nc.sync.dma_start(HOUTS[128 * w : 128 * (w + 1), :],
                                      hrow[:])
    nc.compile()
    return nc


# ------------------------------------------------------------------ launch 3


def _build_launch3(KT):
    NS = KT * 128
    nc = bacc.Bacc("TRN2", target_bir_lowering=False, debug=False,
                   num_devices=NCORES)
    din = lambda n, s, d=F32: nc.dram_tensor(n, s, d, kind="ExternalInput")
    HG = din("HG", [NWIN, 128, KT, OUT], BF16)
    CM2 = din("CM2", [NWIN, 4, NS], F32)    # e9 | asrcO | atgtO | 0
    S3D = din("S3D", [NWIN, 128, NDMA3, 128], BF16)   # first NDMA3 chunks
    SRCWF = din("SRCWF", [128, NWIN, KT], F32)
    OUTT = nc.dram_tensor("OUTT", [NPC, OUT], F32, kind="ExternalOutput")

    with tile.TileContext(nc) as tc:
        with tc.tile_pool(name="const", bufs=1) as cpool:
            e4 = cpool.tile([4, 1], F32)
            nc.gpsimd.memset(e4[:], 1.0)
            slp = cpool.tile([128, 1], F32)
            nc.gpsimd.memset(slp[:], SLOPE)
            iota_bf = cpool.tile([128, 128], BF16)
            nc.gpsimd.iota(iota_bf[:], pattern=[[1, 128]], base=0,
                           channel_multiplier=0,
                           allow_small_or_imprecise_dtypes=True)
            srcwf = cpool.tile([128, NWIN, KT], F32)
            nc.sync.dma_start(srcwf[:], SRCWF[:])
            hall = cpool.tile([128, NWIN, OUT], F32)
            with (
                tc.tile_pool(name="w", bufs=3) as wp,
                tc.tile_pool(name="wv", bufs=6) as wv,
                tc.tile_pool(name="ws_ps", bufs=2, space="PSUM") as wsp,
                tc.tile_pool(name="wn_ps", bufs=2, space="PSUM") as wnp,
                tc.tile_pool(name="wd_ps", bufs=2, space="PSUM") as wdp,
            ):
                for w in range(NWIN):
                    hg = wp.tile([128, KT, OUT], BF16, tag="hg")
                    nc.sync.dma_start(hg[:], HG[w])
                    cm2 = wp.tile([4, NS], F32, tag="cm2")
                    nc.sync.dma_start(cm2[:], CM2[w])
                    s3 = wp.tile([128, KT, 128], BF16, tag="s3")
                    nc.sync.dma_start(s3[:, :NDMA3, :], S3D[w])
                    # build remaining one-hot chunks on DVE / Pool
                    for c in range(NDMA3, KT):
                        if c % 2 == 0:
                            nc.vector.tensor_scalar(
                                s3[:, c, :], iota_bf[:],
                                srcwf[:, w, c : c + 1], None,
                                op0=OP.is_equal)
                        else:
                            nc.gpsimd.tensor_scalar(
                                s3[:, c, :], iota_bf[:],
                                srcwf[:, w, c : c + 1], None,
                                op0=OP.is_equal)
                    s1_ps = wsp.tile([128, KT], F32, tag="s1")
                    for c in range(KT):
                        nc.tensor.matmul(s1_ps[:, c : c + 1],
                                         lhsT=cm2[:, 128 * c : 128 * (c + 1)],
                                         rhs=e4[:], start=True, stop=True)
                    # w1 = max(exp(s), exp(0.01 s))
                    ex1 = wv.tile([128, KT], BF16, tag="ex1")
                    nc.scalar.activation(ex1[:], s1_ps[:], AF.Exp)
                    ex2 = wv.tile([128, KT], BF16, tag="ex2")
                    nc.scalar.activation(ex2[:], s1_ps[:], AF.Exp,
                                         scale=slp[:])
                    w1 = wv.tile([128, KT], BF16, tag="w1")
                    nc.vector.tensor_tensor(out=w1[:], in0=ex1[:],
                                            in1=ex2[:], op=OP.max)
                    V1 = wv.tile([128, KT, OUT], BF16, tag="V1")
                    h3 = KT // 3
                    nc.vector.tensor_tensor(
                        out=V1[:, : 2 * h3, :], in0=hg[:, : 2 * h3, :],
                        in1=w1[:, : 2 * h3].to_broadcast(
                            [128, 2 * h3, OUT]), op=OP.mult)
                    nc.gpsimd.tensor_tensor(
                        out=V1[:, 2 * h3 :, :], in0=hg[:, 2 * h3 :, :],
                        in1=w1[:, 2 * h3 :].to_broadcast(
                            [128, KT - 2 * h3, OUT]),
                        op=OP.mult)
                    num_ps = wnp.tile([128, OUT], F32, tag="num")
                    den_ps = wdp.tile([128, 1], F32, tag="den")
                    for c in range(KT):
                        nc.tensor.matmul(num_ps[:], lhsT=s3[:, c, :],
                                         rhs=V1[:, c, :],
                                         start=(c == 0), stop=(c == KT - 1))
                        nc.tensor.matmul(den_ps[:], lhsT=s3[:, c, :],
                                         rhs=w1[:, c : c + 1],
                                         start=(c == 0), stop=(c == KT - 1))
                    den = wv.tile([128, 1], F32, tag="dens")
                    nc.vector.tensor_scalar(den[:], den_ps[:], 1e-16, None,
                                            op0=OP.add)
                    nc.vector.reciprocal(den[:], den[:])
                    h2 = wv.tile([128, OUT], F32, tag="h2")
                    nc.vector.tensor_scalar(h2[:], num_ps[:], den[:], None,
                                            op0=OP.mult)
                    m0 = wv.tile([128, OUT], F32, tag="m0")
                    nc.vector.tensor_scalar_min(m0[:], h2[:], 0.0)
                    nc.scalar.activation(m0[:], m0[:], AF.Exp)
                    r0 = wv.tile([128, OUT], F32, tag="r0")
                    nc.vector.tensor_scalar(r0[:], h2[:], 0.0, -1.0,
                                            op0=OP.max, op1=OP.add)
                    nc.vector.tensor_tensor(out=hall[:, w, :], in0=m0[:],
                                            in1=r0[:], op=OP.add)
            with tc.tile_pool(name="fin", bufs=1) as fin:
                ex = fin.tile([128, NWIN, OUT], F32)
                nc.scalar.activation(ex[:], hall[:], AF.Exp)
                sm = fin.tile([128, NWIN], F32)
                nc.vector.tensor_reduce(sm[:], ex[:], axis=AX.X, op=OP.add)
                nc.scalar.activation(sm[:], sm[:], AF.Ln)
                res = fin.tile([128, NWIN, OUT], F32)
                nc.vector.tensor_tensor(
                    out=res[:], in0=hall[:],
                    in1=sm[:].to_broadcast([128, NWIN, OUT]), op=OP.subtract)
                nc.sync.dma_start(
                    OUTT[:].rearrange("(w p) f -> p w f", p=128), res[:])
    nc.compile()
    return nc


# ------------------------------------------------------------------ driver


def kernel(X, edge_attr, w_node, b_node, g_node, beta_node,
           w_edge, b_edge, g_edge, beta_edge,
           gat_W, gat_a, out_W, out_a,
           edge_index, matched_car_infra_nodes):
    import ml_dtypes
    import time as _time

    bf = lambda a: np.ascontiguousarray(np.asarray(a, np.float32)).astype(
        ml_dtypes.bfloat16)
    f32 = lambda a: np.ascontiguousarray(np.asarray(a, np.float32))

    X = f32(X)
    ea = f32(edge_attr)
    w_node = f32(w_node); b_node = f32(b_node); g_node = f32(g_node)
    beta_node = f32(beta_node)
    w_edge = f32(w_edge); b_edge = f32(b_edge); g_edge = f32(g_edge)
    beta_edge = f32(beta_edge)
    gW = f32(gat_W); ga = f32(gat_a); oW = f32(out_W); oa = f32(out_a)
    assert np.abs(beta_node).max() < 1e-6 and np.abs(beta_edge).max() < 1e-6

    per_core, pnode, origin, KT, srcw_of, ptgt = _prep(edge_index)
    NS = KT * 128
    NCHE = NWIN * KT
    NCHE4 = ((NCHE + 15) // 16) * 16

    # ---- LN-folded weights (centered + Cholesky u-columns)
    def fold(Wb, bb, g, kdim):
        Wfull = np.concatenate([Wb, bb[None, :]], 0)          # [k, 64]
        m = Wfull.mean(axis=1)                                 # [k]
        Wc = Wfull - m[:, None]
        M = Wc @ Wc.T + 1e-10 * np.eye(kdim)
        B = np.linalg.cholesky(M) / np.sqrt(EMB)
        return np.concatenate([Wc * g[None, :], B], 1)         # [k, 64+k]

    WNC = bf(fold(w_node, b_node, g_node, NODE_DIM + 1))
    WEC = bf(fold(w_edge, b_edge, g_edge, EA_DIM + 1))
    WAB = np.zeros((EMB, 16), np.float32)
    for i in range(HEADS):
        WAB[:, i] = gW[i] @ ga[i, :OUT]
        WAB[:, 8 + i] = gW[i] @ ga[i, OUT : 2 * OUT]
    WAB = bf(WAB)
    AE9 = np.zeros((EMB, 16), np.float32)
    for i in range(HEADS):
        AE9[:, i] = ga[i, 2 * OUT :]
    AE9[:, 8] = oa[2 * OUT :]
    A2 = np.zeros((2, 128, 16), np.float32)
    A2[0, :EMB] = AE9
    A2[1, EMB:] = AE9
    AE9 = bf(A2.transpose(1, 0, 2))

    # ---- launch 1 inputs
    Xp = np.zeros((NPN, NODE_DIM + 1), np.float32)
    valid = origin >= 0
    Xp[valid, :NODE_DIM] = X[origin[valid]]
    Xp[:, NODE_DIM] = 1.0
    src = np.asarray(edge_index[0]).astype(np.int64)

    in_maps1 = []
    for k in range(NCORES):
        eslot = per_core[k]                                    # [NWIN, NS]
        eat = np.zeros((NCHE4 * 128, EA_DIM + 1), np.float32)
        es = eslot.reshape(-1)
        m = es >= 0
        eat[: NS * NWIN][m, :EA_DIM] = ea[es[m]]
        eat[: NS * NWIN][m, EA_DIM] = 1.0
        in_maps1.append(dict(
            XT17=bf(Xp[k * NPC : (k + 1) * NPC].T),
            WNC=WNC, WAB=WAB, AE9=AE9,
            EAT9=bf(eat.T), WEC=WEC))

    nc1 = _build_launch1(NCHE4)
    kernel.nc1 = nc1
    _t = _time.perf_counter()
    res1 = run_bass_kernel_spmd(nc1, in_maps1, core_ids=list(range(NCORES)))
    kernel.wall1 = _time.perf_counter() - _t

    # ---- host: assemble tables, gather per-slot inputs for launch 2
    XF = np.zeros((NPN, EMB), ml_dtypes.bfloat16)
    ADF = np.zeros((NPN, 16), np.float32)
    ESCF = []
    for k in range(NCORES):
        XF[k * NPC : (k + 1) * NPC] = res1.results[k]["XO"]
        ADF[k * NPC : (k + 1) * NPC] = res1.results[k]["AD"]
        # ESC9 [128, NCHE4, 9] -> slot-major [NWIN, NS, 9]
        e9 = np.asarray(res1.results[k]["ESC9"], np.float32)[:, :NCHE, :]
        e9 = e9.transpose(1, 0, 2).reshape(NWIN, NS, 9)
        ESCF.append(e9)

    # one-hot S3 per core (shared by launches 2 and 3)
    in_maps2 = []
    s3_cores = []
    for k in range(NCORES):
        eslot = per_core[k]
        es = eslot.reshape(NWIN, NS)
        m = es >= 0
        tgtrow = np.zeros((NWIN, NS), np.int64)
        tgtrow[m] = ptgt[es[m]]
        srcw = np.full((NWIN, NS), -1, np.int64)
        srcw[m] = srcw_of[es[m]]

        XGk = np.zeros((NWIN, NS, EMB), ml_dtypes.bfloat16)
        XGk[m] = XF[tgtrow[m]]
        CMTk = np.zeros((NWIN, 24, NS), np.float32)
        CMTk[:, 0:8, :] = ESCF[k][:, :, 0:8].transpose(0, 2, 1)
        srcrow_k = np.zeros((NWIN, NS), np.int64)
        # src row = core base + win*128 + srcw
        wid = np.arange(NWIN)[:, None]
        srcrow_k[m] = (k * NPC + (wid + np.zeros_like(srcw))[m] * 128
                       + srcw[m])
        asrc = np.zeros((NWIN, NS, 8), np.float32)
        asrc[m] = ADF[srcrow_k[m], 0:8]
        atgt = np.zeros((NWIN, NS, 8), np.float32)
        atgt[m] = ADF[tgtrow[m], 8:16]
        CMTk[:, 8:16, :] = asrc.transpose(0, 2, 1)
        CMTk[:, 16:24, :] = atgt.transpose(0, 2, 1)

        S3k = np.zeros((NWIN, NS, 128), ml_dtypes.bfloat16)
        ww, ss = np.nonzero(m)
        S3k[ww, ss, srcw[ww, ss]] = 1.0
        S3k = S3k.reshape(NWIN, KT, 128, 128).transpose(0, 2, 1, 3)
        s3_cores.append(np.ascontiguousarray(S3k))

        WB4 = np.zeros((128, 4, 128), np.float32)
        for j in range(4):
            for il in range(2):
                WB4[64 * il : 64 * il + 64, j,
                    64 * il : 64 * il + 64] = gW[2 * j + il]
        E24 = np.zeros((24, 8), np.float32)
        for i in range(8):
            E24[i, i] = 1.0
            E24[8 + i, i] = 1.0
            E24[16 + i, i] = 1.0
        OWC = np.zeros((512, 66), np.float32)
        OWC[:, 0:64] = oW
        OWC[:, 64] = oW @ oa[:OUT]
        OWC[:, 65] = oW @ oa[OUT : 2 * OUT]
        in_maps2.append(dict(
            XG=_slotmaj(XGk, KT, EMB),
            CMT=CMTk,
            S3H=s3_cores[k],
            E24=E24,
            OWC=bf(np.ascontiguousarray(
                OWC.reshape(4, 128, 66).transpose(1, 0, 2))),
            WB4=bf(WB4),
        ))

    nc2 = _build_launch2(KT)
    kernel.nc2 = nc2
    _t = _time.perf_counter()
    res2 = run_bass_kernel_spmd(nc2, in_maps2, core_ids=list(range(NCORES)))
    kernel.wall2 = _time.perf_counter() - _t

    # ---- host: assemble h_out table, gather for launch 3
    HF = np.zeros((NPN, 66), np.float32)
    for k in range(NCORES):
        HF[k * NPC : (k + 1) * NPC] = res2.results[k]["HOUTS"]
    HFb = HF[:, 0:64].astype(ml_dtypes.bfloat16)

    in_maps3 = []
    for k in range(NCORES):
        eslot = per_core[k]
        es = eslot.reshape(NWIN, NS)
        m = es >= 0
        tgtrow = np.zeros((NWIN, NS), np.int64)
        tgtrow[m] = ptgt[es[m]]
        srcw = np.full((NWIN, NS), -1, np.int64)
        srcw[m] = srcw_of[es[m]]
        wid = np.arange(NWIN)[:, None]
        srcrow_k = np.zeros((NWIN, NS), np.int64)
        srcrow_k[m] = (k * NPC + (wid + np.zeros_like(srcw))[m] * 128
                       + srcw[m])

        HGk = np.zeros((NWIN, NS, OUT), ml_dtypes.bfloat16)
        HGk[m] = HFb[tgtrow[m]]
        CM2k = np.zeros((NWIN, 4, NS), np.float32)
        CM2k[:, 0, :] = ESCF[k][:, :, 8]
        a_s = np.zeros((NWIN, NS), np.float32)
        a_s[m] = HF[srcrow_k[m], 64]
        a_t = np.zeros((NWIN, NS), np.float32)
        a_t[m] = HF[tgtrow[m], 65]
        CM2k[:, 1, :] = a_s
        CM2k[:, 2, :] = a_t
        srcwf_f = srcw.reshape(NWIN, KT, 128).transpose(2, 0, 1).astype(
            np.float32)
        in_maps3.append(dict(
            HG=_slotmaj(HGk, KT, OUT),
            CM2=CM2k,
            S3D=np.ascontiguousarray(s3_cores[k][:, :, :NDMA3, :]),
            SRCWF=np.ascontiguousarray(srcwf_f)))

    nc3 = _build_launch3(KT)
    kernel.nc3 = nc3
    _t = _time.perf_counter()
    res3 = run_bass_kernel_spmd(nc3, in_maps3, core_ids=list(range(NCORES)))
    kernel.wall3 = _time.perf_counter() - _t

    outp = np.zeros((NPN, OUT), np.float32)
    for k in range(NCORES):
        outp[k * NPC : (k + 1) * NPC] = res3.results[k]["OUTT"]
    out = np.zeros((N, OUT), np.float32)
    valid = origin >= 0
    out[origin[valid]] = outp[valid]
    return out


def _slotmaj(A, KT, F):
    """[NWIN, NS, F] with slot s=(c*128+p) -> [NWIN, 128, KT, F]."""
    NW = A.shape[0]
    return np.ascontiguousarray(
        A.reshape(NW, KT, 128, F).transpose(0, 2, 1, 3))
